# revision 1
# baseline (speedup 1.0000x reference)
"""Single transformer block on 8 NeuronCores.

Sharding: core c handles batch b=c//2, sequence half c%2 (T=1024 tokens).
All token-wise ops (LN, QKV, c_proj, MLP) are purely local; attention needs
the full sequence of K/V per batch, obtained with a pairwise AllGather
between cores {2b, 2b+1}.

Layout strategy (per core):
  - residual stream x: token-major [128t x D] SBUF tiles
  - h^T, m^T: feature-major (PE transpose) so matmul lhsT/rhs slices need
    no further transposes
  - q^T, k^T: feature-major straight out of the QKV matmul (lhsT = W_attn)
  - v: token-major straight out of the QKV matmul (lhsT = h^T)
  - scores computed transposed S^T[k, q] so the softmax denominator comes
    free from an appended ones-column in V during the AV matmul
  - causal mask applied as a 0/1 multiply on P=exp(S) (exp of a masked
    score is finite; multiply-by-zero afterwards is exact)
All matmuls run as float32r (full PE rate) except mproj which runs bf16 to
halve the 16.8MB gelu-activation buffer.
"""

import math
from contextlib import ExitStack

import numpy as np
import ml_dtypes

import concourse.bacc as bacc
import concourse.bass as bass
import concourse.mybir as mybir
import concourse.tile as tile
from concourse.masks import make_identity

F32 = mybir.dt.float32
F32R = mybir.dt.float32r
BF16 = mybir.dt.bfloat16
AF = mybir.ActivationFunctionType
ALU = mybir.AluOpType

EPS = 1e-5


def r(ap):
    return ap.bitcast(F32R)


class Cfg:
    def __init__(self, B=4, S=2048, D=1024, H=16, F=4096, n_cores=8,
                 bs=256, attn_bf16=True):
        self.B, self.S, self.D, self.H, self.F = B, S, D, H, F
        self.n_cores = n_cores
        assert n_cores == 2 * B
        self.HD = D // H
        assert self.HD == 64
        self.T = S // 2            # tokens per core
        self.TB = self.T // 128    # token 128-blocks
        self.DC = D // 128         # contraction chunks over D
        self.QF = min(512, self.T)  # q free-dim tile
        self.QH = self.T // self.QF
        self.KC = S // 128         # key 128-chunks over full sequence
        self.VF = min(512, D)      # out-feature tile for token-major outs
        self.FH = D // self.VF
        self.GB = F // 128         # MLP hidden 128-blocks
        self.HPB = 128 // self.HD  # heads per 128-feature block (=2)
        self.BS = min(bs, self.T)  # stripe block (q-slot) size
        self.SLOTS = self.T // self.BS
        self.KCH = self.KC // 2    # AG chunks per parity block
        self.CPB = self.BS // 128  # 128-chunks per stripe block
        self.native_gelu = True
        self.attn_bf16 = attn_bf16
        self.debug_taps = False


def chunk_absblk(c, kc):
    # absolute stripe-block index covered by AG chunk kc
    parity = kc // c.KCH
    loc = kc % c.KCH
    return 2 * ((loc * 128) // c.BS) + parity


def build(cfg: Cfg):
    c = cfg
    KVT = BF16 if c.attn_bf16 else F32R
    VBT = BF16 if c.attn_bf16 else F32
    nc = bacc.Bacc(None, target_bir_lowering=False)

    # ---------------- I/O ----------------
    x_in = nc.dram_tensor("x", [c.T, c.D], F32, kind="ExternalInput")
    w_attn = nc.dram_tensor("w_attn", [c.D, 3 * c.D], F32R, kind="ExternalInput")
    w_cproj = nc.dram_tensor("w_cproj", [c.D, c.D], F32R, kind="ExternalInput")
    w_fc = nc.dram_tensor("w_fc", [c.D, c.F], F32R, kind="ExternalInput")
    w_mproj = nc.dram_tensor("w_mproj", [c.F, c.D], BF16, kind="ExternalInput")
    ln1w_in = nc.dram_tensor("ln1w", [1, c.D], F32, kind="ExternalInput")
    ln1b_in = nc.dram_tensor("ln1b", [1, c.D], F32, kind="ExternalInput")
    ln2w_in = nc.dram_tensor("ln2w", [1, c.D], F32, kind="ExternalInput")
    ln2b_in = nc.dram_tensor("ln2b", [1, c.D], F32, kind="ExternalInput")
    battn_qk_in = nc.dram_tensor("battn_qk", [128, 2 * c.DC], F32,
                                 kind="ExternalInput")
    bv_in = nc.dram_tensor("bv", [1, c.D], F32, kind="ExternalInput")
    bcp_in = nc.dram_tensor("bcp", [1, c.D], F32, kind="ExternalInput")
    bmp_in = nc.dram_tensor("bmp", [1, c.D], F32, kind="ExternalInput")
    bfc_in = nc.dram_tensor("bfc", [128, c.GB], F32, kind="ExternalInput")
    qidx_in = nc.dram_tensor("qidx", [1, c.T], F32, kind="ExternalInput")
    kofs_in = nc.dram_tensor("kofs", [128, c.KC], F32, kind="ExternalInput")
    y_out = nc.dram_tensor("y", [c.T, c.D], F32, kind="ExternalOutput")
    if c.debug_taps:
        dbg_k = nc.dram_tensor("dbg_k", [2 * c.D, c.T], F32, kind="ExternalOutput")
        dbg_v = nc.dram_tensor("dbg_v", [2 * c.T, c.D], F32, kind="ExternalOutput")
        dbg_q = nc.dram_tensor("dbg_q", [128, c.T], F32, kind="ExternalOutput")
        dbg_at = nc.dram_tensor("dbg_at", [128, c.T], F32, kind="ExternalOutput")

    pairs = [[2 * b, 2 * b + 1] for b in range(c.B)]

    def bcast(dram, p=128):
        # partition-broadcast DMA source: read row 0 for every partition
        return bass.AP(tensor=dram, offset=0, ap=[[0, p], [1, dram.shape[1]]])

    with tile.TileContext(nc) as tc, ExitStack() as es:
        dpool = es.enter_context(tc.tile_pool(name="dram", bufs=1, space="DRAM"))
        gconst = es.enter_context(tc.tile_pool(name="gconst", bufs=1))

        # DRAM bounce buffers for the pairwise AllGather
        flag_loc = dpool.tile([1, 16], F32)
        flag_out = dpool.tile([1, 16], F32)
        kb_loc = dpool.tile([c.D, c.T], KVT)
        kb_full = dpool.tile([2 * c.D, c.T], KVT)
        vb_loc = dpool.tile([c.T, c.D], VBT)
        vb_full = dpool.tile([2 * c.T, c.D], VBT)

        # ---------------- global constants ----------------
        ident = gconst.tile([128, 128], F32)
        make_identity(nc, ident[:])
        eps_t = gconst.tile([128, 1], F32)
        nc.vector.memset(eps_t[:], EPS)
        ones64_f = gconst.tile([1, 64], F32)
        nc.vector.memset(ones64_f[:], 1.0)
        nc.sync.dma_start(out=flag_loc[:], in_=ones64_f[0:1, 0:16])
        ones64 = gconst.tile([1, 64], F32R)
        nc.vector.tensor_copy(ones64[:], ones64_f[:])

        def layernorm_t(src_tiles, w, b, out_pool, out_tag):
            """token-major LN over free axis + transpose to feature-major.

            Returns DC tiles of [128 d-features, T]."""
            outs = []
            for i in range(c.DC):
                ot = out_pool.tile([128, c.T], F32R, tag=f"{out_tag}{i}",
                                   name=f"{out_tag}{i}")
                outs.append(ot)
            with (
                tc.tile_pool(name=f"ln_{out_tag}", bufs=3) as lnp,
                tc.tile_pool(name=f"ps_tr_{out_tag}", bufs=3,
                             space="PSUM") as ps_tr,
            ):
                for tb in range(c.TB):
                    src = src_tiles[tb]
                    nsg = c.D // 512 if c.D % 512 == 0 else 1
                    sgw = c.D // nsg
                    st = lnp.tile([128, nsg, 6], F32, tag="st")
                    for sg in range(nsg):
                        nc.vector.bn_stats(
                            out=st[:, sg, :],
                            in_=src[:, sg * sgw:(sg + 1) * sgw])
                    mv = lnp.tile([128, 2], F32, tag="mv")
                    nc.vector.bn_aggr(out=mv[:], in_=st[:])
                    sd = lnp.tile([128, 1], F32, tag="sd")
                    nc.scalar.activation(sd[:], mv[:, 1:2], AF.Sqrt,
                                         bias=eps_t[:, 0:1])
                    rs = lnp.tile([128, 1], F32, tag="rs")
                    nc.vector.reciprocal(rs[:], sd[:])
                    ht_ = lnp.tile([128, c.D], F32, tag="h")
                    nc.vector.tensor_scalar(
                        out=ht_[:], in0=src[:], scalar1=mv[:, 0:1],
                        scalar2=rs[:, 0:1], op0=ALU.subtract, op1=ALU.mult)
                    nc.vector.tensor_mul(ht_[:], ht_[:], w[:])
                    nc.vector.tensor_add(ht_[:], ht_[:], b[:])
                    for i in range(c.DC):
                        pt = ps_tr.tile([128, 128], F32, tag="tr")
                        nc.tensor.transpose(
                            pt[:], ht_[:, i * 128:(i + 1) * 128], ident[:])
                        nc.scalar.activation(
                            outs[i][:, tb * 128:(tb + 1) * 128], pt[:],
                            AF.Identity)
            return outs

        # ================= phase A: LN1 + QKV =================
        es_x = ExitStack()
        xpool = es_x.enter_context(tc.tile_pool(name="xpool", bufs=1, side="left"))
        xt = []
        for tb in range(c.TB):
            t = xpool.tile([128, c.D], F32, tag=f"x{tb}", name=f"x{tb}")
            nc.sync.dma_start(out=t[:], in_=x_in[tb * 128:(tb + 1) * 128, :])
            xt.append(t)

        es_qt = ExitStack()
        qtpool = es_qt.enter_context(tc.tile_pool(name="qtpool", bufs=1, side="right"))
        qtp = []
        for j in range(c.H // c.HPB):
            qtp.append(qtpool.tile([128, c.T], KVT, tag=f"qt{j}",
                                   name=f"qt{j}"))

        with (
            tc.tile_pool(name="aconst", bufs=1) as aconst,
            tc.tile_pool(name="htp", bufs=1) as htpool,
        ):
            ln1w = aconst.tile([128, c.D], F32)
            ln1b = aconst.tile([128, c.D], F32)
            bv_b = aconst.tile([128, c.D], F32)
            for t, d in [(ln1w, ln1w_in), (ln1b, ln1b_in), (bv_b, bv_in)]:
                nc.sync.dma_start(out=t[:], in_=bcast(d))
            battn_qk = aconst.tile([128, 2 * c.DC], F32)
            nc.sync.dma_start(out=battn_qk[:], in_=battn_qk_in[:, :])

            ht = layernorm_t(xt, ln1w, ln1b, htpool, "ht")

            with (
                tc.tile_pool(name="wa", bufs=2) as wap,
                tc.tile_pool(name="kout", bufs=3) as kop,
                tc.tile_pool(name="ps_mm", bufs=4, space="PSUM") as psmm,
            ):
                # ---- k^T pass (feature-major, straight to DRAM bounce) ----
                wk_all = wap.tile([128, c.DC, c.D], F32R, tag="wa",
                                  name="wk_all")
                nc.scalar.dma_start(
                    out=wk_all[:],
                    in_=w_attn[:, c.D:2 * c.D].rearrange(
                        "(i p) f -> p i f", p=128))
                for m in range(c.DC):
                    wt = [wk_all[:, i, m * 128:(m + 1) * 128]
                          for i in range(c.DC)]
                    for th in range(c.QH):
                        ps = psmm.tile([128, c.QF], F32, tag="ps")
                        for i in range(c.DC):
                            nc.tensor.matmul(
                                ps[:], wt[i],
                                ht[i][:, th * c.QF:(th + 1) * c.QF],
                                start=(i == 0), stop=(i == c.DC - 1))
                        ko = kop.tile([128, c.QF], KVT, tag="ko")
                        nc.scalar.activation(
                            ko[:], ps[:], AF.Identity,
                            bias=battn_qk[:, c.DC + m:c.DC + m + 1])
                        nc.sync.dma_start(
                            out=kb_loc[m * 128:(m + 1) * 128,
                                       th * c.QF:(th + 1) * c.QF],
                            in_=ko[:])

                # kick the K AllGather as soon as the k-pass is done so it
                # overlaps the v-pass and q-pass
                nc.gpsimd.collective_compute(
                    "AllGather", ALU.bypass, ins=[kb_loc[:]],
                    outs=[kb_full[:]], replica_groups=pairs)

                # ---- v pass (token-major, straight to DRAM bounce) ----
                wv_all = wap.tile([128, c.DC, c.D], F32R, tag="wa",
                                  name="wv_all")
                nc.scalar.dma_start(
                    out=wv_all[:],
                    in_=w_attn[:, 2 * c.D:3 * c.D].rearrange(
                        "(i p) f -> p i f", p=128))
                for vh in range(c.FH):
                    for tb in range(c.TB):
                        ps = psmm.tile([128, c.VF], F32, tag="ps")
                        for i in range(c.DC):
                            nc.tensor.matmul(
                                ps[:], ht[i][:, tb * 128:(tb + 1) * 128],
                                wv_all[:, i, vh * c.VF:(vh + 1) * c.VF],
                                start=(i == 0), stop=(i == c.DC - 1))
                        vo = kop.tile([128, c.VF], VBT, tag="vo")
                        nc.vector.tensor_add(
                            vo[:], ps[:], bv_b[:, vh * c.VF:(vh + 1) * c.VF])
                        nc.sync.dma_start(
                            out=vb_loc[tb * 128:(tb + 1) * 128,
                                       vh * c.VF:(vh + 1) * c.VF],
                            in_=vo[:])

                # ---- pairwise AllGather of v ----
                nc.gpsimd.collective_compute(
                    "AllGather", ALU.bypass, ins=[vb_loc[:]],
                    outs=[vb_full[:]], replica_groups=pairs)
                # Tiny AllReduce as a delivery barrier: its completion
                # requires the peer's contribution, and collectives execute
                # in order per rank, so peer AG data has landed by then.
                cc_bar = nc.gpsimd.collective_compute(
                    "AllReduce", ALU.add, ins=[flag_loc[:]],
                    outs=[flag_out[:]], replica_groups=pairs)

                # ---- q^T pass (feature-major, stays in SBUF) ----
                wq_all = wap.tile([128, c.DC, c.D], F32R, tag="wa",
                                  name="wq_all")
                nc.scalar.dma_start(
                    out=wq_all[:],
                    in_=w_attn[:, 0:c.D].rearrange(
                        "(i p) f -> p i f", p=128))
                for m in range(c.DC):
                    wt = [wq_all[:, i, m * 128:(m + 1) * 128]
                          for i in range(c.DC)]
                    for th in range(c.QH):
                        ps = psmm.tile([128, c.QF], F32, tag="ps")
                        for i in range(c.DC):
                            nc.tensor.matmul(
                                ps[:], wt[i],
                                ht[i][:, th * c.QF:(th + 1) * c.QF],
                                start=(i == 0), stop=(i == c.DC - 1))
                        # scale by 1/sqrt(HD) at eviction (bias pre-scaled)
                        nc.scalar.activation(
                            qtp[m][:, th * c.QF:(th + 1) * c.QF], ps[:],
                            AF.Identity, bias=battn_qk[:, m:m + 1],
                            scale=1.0 / math.sqrt(c.HD))

        # ================= phase B: attention =================
        # prefetch c_proj weights during attention (scalar DMA ring)
        es_wc = ExitStack()
        wcp = es_wc.enter_context(tc.tile_pool(name="wc", bufs=1, side="left"))
        wc_all = wcp.tile([128, c.DC, c.D], F32R, tag="wc", name="wc_all")
        nc.scalar.dma_start(
            out=wc_all[:],
            in_=w_cproj[:, :].rearrange("(i p) f -> p i f", p=128))

        es_at = ExitStack()
        atpool = es_at.enter_context(tc.tile_pool(name="atpool", bufs=1, side="left"))
        at = []
        for j in range(c.DC):
            at.append(atpool.tile([128, c.T], F32R, tag=f"at{j}",
                                  name=f"at{j}"))

        with (
            tc.tile_pool(name="bconst", bufs=1) as bconst,
            tc.tile_pool(name="mask", bufs=1) as maskp,
            tc.tile_pool(name="kv", bufs=3) as kvp,
            tc.tile_pool(name="pt", bufs=c.KC + 2) as ptp,
            tc.tile_pool(name="rec", bufs=2) as recp,
            tc.tile_pool(name="ps_s", bufs=6, space="PSUM") as pss,
            tc.tile_pool(name="ps_o", bufs=2, space="PSUM") as pso,
        ):
            qidx = bconst.tile([128, c.T], F32)
            nc.sync.dma_start(out=qidx[:], in_=bcast(qidx_in))
            kofs = bconst.tile([128, c.KC], F32)
            nc.sync.dma_start(out=kofs[:], in_=kofs_in[:, :])

            # per-slot chunk lists (compile-time causal structure)
            slot_chunks = []
            for sl in range(c.SLOTS):
                cl = [kc for kc in range(c.KC)
                      if chunk_absblk(c, kc) <= 2 * sl + 1]
                slot_chunks.append(cl)

            # group q-slots in pairs: one 512-wide QK/exp per k-chunk
            groups = []
            sl = 0
            while sl < c.SLOTS:
                g = [sl, sl + 1] if sl + 1 < c.SLOTS else [sl]
                groups.append(g)
                sl += len(g)

            # pre-generate boundary masks per (group, chunk) where the chunk
            # may cross the causal diagonal of ANY slot in the group
            masks = {}
            for gi, g in enumerate(groups):
                gw = len(g) * c.BS
                qsl = slice(g[0] * c.BS, g[0] * c.BS + gw)
                for kc in slot_chunks[g[-1]]:
                    if chunk_absblk(c, kc) >= 2 * g[0]:
                        mk = maskp.tile([128, gw], BF16,
                                        tag=f"mk{gi}_{kc}",
                                        name=f"mk{gi}_{kc}")
                        nc.vector.tensor_scalar(
                            out=mk[:], in0=qidx[:, qsl],
                            scalar1=kofs[:, kc:kc + 1], scalar2=None,
                            op0=ALU.is_ge)
                        masks[(gi, kc)] = mk

            from concourse.tile import add_dep_helper
            for jj in range(c.H // c.HPB):
                ktp = kvp.tile([128, c.S], KVT, tag="ktp")
                for hp in range(c.HPB):
                    hh = c.HPB * jj + hp
                    psl = slice(hp * 64, hp * 64 + 64)
                    d1 = nc.sync.dma_start(
                        out=ktp[psl, 0:c.T],
                        in_=kb_full[64 * hh:64 * hh + 64, :])
                    d2 = nc.sync.dma_start(
                        out=ktp[psl, c.T:c.S],
                        in_=kb_full[c.D + 64 * hh:c.D + 64 * hh + 64, :])
                    add_dep_helper(d1.ins, cc_bar.ins,
                                   reason="AG delivery barrier")
                    add_dep_helper(d2.ins, cc_bar.ins,
                                   reason="AG delivery barrier")
                vt_f2 = kvp.tile([128, c.KC, c.HPB * 64], VBT,
                                 tag="vtf")
                dv = nc.sync.dma_start(
                    out=vt_f2[:],
                    in_=vb_full[:, 128 * jj:128 * jj + 128].rearrange(
                        "(kc p) f -> p kc f", p=128))
                add_dep_helper(dv.ins, cc_bar.ins,
                               reason="AG delivery barrier")
                for hp in range(c.HPB):
                    h = c.HPB * jj + hp
                    base = hp * 64
                    vt = kvp.tile([128, c.KC, 65], BF16, tag="vt")
                    nc.vector.tensor_copy(
                        vt[:, :, 0:64], vt_f2[:, :, hp * 64:hp * 64 + 64])
                    nc.vector.memset(vt[:, :, 64:65], 1.0)

                    for gi, g in enumerate(groups):
                        gw = len(g) * c.BS
                        gq = slice(g[0] * c.BS, g[0] * c.BS + gw)
                        rhs_q = qtp[jj][base:base + 64, gq]
                        cl_all = slot_chunks[g[-1]]
                        pt_of = {}
                        for kc in cl_all:
                            # chunks needed only by the upper slot of the
                            # pair: compute just that 256-wide half
                            diff = (len(g) == 2 and
                                    chunk_absblk(c, kc) > 2 * g[0] + 1)
                            w = c.BS if diff else gw
                            rq = (qtp[jj][base:base + 64,
                                          g[1] * c.BS:(g[1] + 1) * c.BS]
                                  if diff else rhs_q)
                            ps = pss.tile([128, gw], F32, tag="s")
                            nc.tensor.matmul(
                                ps[:, 0:w],
                                ktp[base:base + 64,
                                    kc * 128:(kc + 1) * 128],
                                rq, start=True, stop=True)
                            pt = ptp.tile([128, gw], BF16, tag="pt")
                            nc.scalar.activation(pt[:, 0:w], ps[:, 0:w],
                                                 AF.Exp)
                            if (gi, kc) in masks:
                                mw = masks[(gi, kc)]
                                msl = (slice(c.BS, 2 * c.BS) if diff
                                       else slice(0, gw))
                                nc.vector.tensor_mul(
                                    pt[:, 0:w], pt[:, 0:w], mw[:, msl])
                            pt_of[kc] = (pt, diff)
                        for half, sl in enumerate(g):
                            qsl = slice(sl * c.BS, (sl + 1) * c.BS)
                            cl = slot_chunks[sl]
                            po = pso.tile([65, c.BS], F32, tag="o")
                            for n, kc in enumerate(cl):
                                pt, diff = pt_of[kc]
                                col = 0 if diff else half * c.BS
                                nc.tensor.matmul(
                                    po[:], vt[:, kc, :],
                                    pt[:, col:col + c.BS],
                                    start=(n == 0),
                                    stop=(n == len(cl) - 1))
                            # normalize by softmax denominator (row 64)
                            rec = recp.tile([1, c.BS], F32R, tag="rec")
                            with nc.allow_low_precision(
                                    reason="softmax denom in f32r"):
                                nc.vector.reciprocal(rec[:], po[64:65, :])
                            bc = pss.tile([64, c.BS], F32, tag="s")
                            nc.tensor.matmul(bc[:], ones64[:], rec[:],
                                             start=True, stop=True)
                            bcs = recp.tile([64, c.BS], F32, tag="bcs")
                            nc.vector.tensor_copy(bcs[:], bc[:])
                            nc.vector.tensor_mul(
                                at[jj][base:base + 64, qsl], po[0:64, :],
                                bcs[:])

        if c.debug_taps:
            with tc.tile_pool(name="dbgp", bufs=2) as dbgp:
                for blk in range(2 * c.D // 128):
                    st_ = dbgp.tile([128, c.T], KVT, tag="st")
                    nc.sync.dma_start(out=st_[:], in_=kb_full[blk * 128:(blk + 1) * 128, :])
                    ft_ = dbgp.tile([128, c.T], F32, tag="ft")
                    nc.vector.tensor_copy(ft_[:], st_[:])
                    nc.sync.dma_start(out=dbg_k[blk * 128:(blk + 1) * 128, :], in_=ft_[:])
                for blk in range(2 * c.T // 128):
                    st_ = dbgp.tile([128, c.D], VBT, tag="st2")
                    nc.sync.dma_start(out=st_[:], in_=vb_full[blk * 128:(blk + 1) * 128, :])
                    ft_ = dbgp.tile([128, c.D], F32, tag="ft2")
                    nc.vector.tensor_copy(ft_[:], st_[:])
                    nc.sync.dma_start(out=dbg_v[blk * 128:(blk + 1) * 128, :], in_=ft_[:])
                qf_ = dbgp.tile([128, c.T], F32, tag="qf")
                nc.vector.tensor_copy(qf_[:], qtp[0][:])
                nc.sync.dma_start(out=dbg_q[:], in_=qf_[:])
                af_ = dbgp.tile([128, c.T], F32, tag="af")
                nc.vector.tensor_copy(af_[:], at[0][:])
                nc.sync.dma_start(out=dbg_at[:], in_=af_[:])

        es_qt.close()

        # ================= phase C: c_proj + residual =================
        es_x2 = ExitStack()
        x2pool = es_x2.enter_context(tc.tile_pool(name="x2pool", bufs=1, side="right"))
        x2t = []
        with (
            tc.tile_pool(name="cconst", bufs=1) as cconst,
            tc.tile_pool(name="ps_c", bufs=4, space="PSUM") as psc,
        ):
            bcp_b = cconst.tile([128, c.D], F32)
            nc.sync.dma_start(out=bcp_b[:], in_=bcast(bcp_in))
            for tb in range(c.TB):
                x2 = x2pool.tile([128, c.D], F32, tag=f"x2_{tb}",
                                 name=f"x2_{tb}")
                for fh in range(c.FH):
                    fsl = slice(fh * c.VF, (fh + 1) * c.VF)
                    ps = psc.tile([128, c.VF], F32, tag="ps")
                    for i in range(c.DC):
                        nc.tensor.matmul(
                            ps[:], at[i][:, tb * 128:(tb + 1) * 128],
                            wc_all[:, i, fh * c.VF:(fh + 1) * c.VF],
                            start=(i == 0), stop=(i == c.DC - 1))
                    nc.vector.tensor_add(x2[:, fsl], ps[:], xt[tb][:, fsl])
                    nc.vector.tensor_add(x2[:, fsl], x2[:, fsl],
                                         bcp_b[:, fsl])
                x2t.append(x2)

        es_at.close()
        es_wc.close()
        es_x.close()

        # ================= phase D: LN2 + MLP =================
        with (
            tc.tile_pool(name="dconst", bufs=1) as dconst,
            tc.tile_pool(name="gt", bufs=1) as gtp,
        ):
            ln2w = dconst.tile([128, c.D], F32)
            ln2b = dconst.tile([128, c.D], F32)
            bmp_b = dconst.tile([128, c.D], F32)
            for t, d in [(ln2w, ln2w_in), (ln2b, ln2b_in), (bmp_b, bmp_in)]:
                nc.sync.dma_start(out=t[:], in_=bcast(d))
            bfc = dconst.tile([128, c.GB], F32)
            nc.sync.dma_start(out=bfc[:], in_=bfc_in[:, :])

            gt = []
            with tc.tile_pool(name="mtp", bufs=1) as mtpool:
                mt = layernorm_t(x2t, ln2w, ln2b, mtpool, "mt")

                # ---------------- fc + gelu ----------------
                with (
                    tc.tile_pool(name="wf", bufs=2) as wfp,
                    tc.tile_pool(name="gw", bufs=3) as gw,
                    tc.tile_pool(name="ps_g", bufs=4, space="PSUM") as psg,
                ):
                    c1 = math.sqrt(2.0 / math.pi)
                    c2 = 0.044715
                    GPW = 512 // 128  # g-blocks per W_fc slab
                    wf_all = None
                    for gb in range(c.GB):
                        if gb % GPW == 0:
                            wf_all = wfp.tile([128, c.DC, 512], F32R,
                                              tag="wf", name=f"wf{gb}")
                            j = gb // GPW
                            nc.scalar.dma_start(
                                out=wf_all[:],
                                in_=w_fc[:, j * 512:(j + 1) * 512].rearrange(
                                    "(i p) f -> p i f", p=128))
                        gl = (gb % GPW) * 128
                        wt = [wf_all[:, i, gl:gl + 128] for i in range(c.DC)]
                        g = gtp.tile([128, c.T], BF16, tag=f"g{gb}",
                                     name=f"g{gb}")
                        for th in range(c.QH):
                            tsl = slice(th * c.QF, (th + 1) * c.QF)
                            ps = psg.tile([128, c.QF], F32, tag="ps")
                            for i in range(c.DC):
                                nc.tensor.matmul(
                                    ps[:], wt[i],
                                    mt[i][:, th * c.QF:(th + 1) * c.QF],
                                    start=(i == 0), stop=(i == c.DC - 1))
                            if c.native_gelu:
                                nc.scalar.activation(
                                    g[:, tsl], ps[:], AF.Gelu_apprx_tanh,
                                    bias=bfc[:, gb:gb + 1])
                            else:
                                # 0.5*u*(1+tanh(c1*(u+c2*u^3)))
                                u = gw.tile([128, c.QF], F32, tag="u")
                                nc.scalar.activation(
                                    u[:], ps[:], AF.Identity,
                                    bias=bfc[:, gb:gb + 1])
                                s1 = gw.tile([128, c.QF], F32, tag="s1")
                                nc.vector.tensor_mul(s1[:], u[:], u[:])
                                nc.vector.tensor_scalar(
                                    out=s1[:], in0=s1[:], scalar1=c1 * c2,
                                    scalar2=c1, op0=ALU.mult, op1=ALU.add)
                                nc.vector.tensor_mul(s1[:], s1[:], u[:])
                                nc.scalar.activation(s1[:], s1[:], AF.Tanh)
                                nc.vector.tensor_scalar(
                                    out=s1[:], in0=s1[:], scalar1=1.0,
                                    scalar2=0.5, op0=ALU.add, op1=ALU.mult)
                                nc.vector.tensor_mul(g[:, tsl], s1[:], u[:])
                        gt.append(g)

            # ---------------- mproj + residual ----------------
            with (
                tc.tile_pool(name="wm", bufs=1) as wmp,
                tc.tile_pool(name="yout", bufs=3) as yop,
                tc.tile_pool(name="ps_m", bufs=4, space="PSUM") as psm,
            ):
                wm_all = []
                for fh in range(c.FH):
                    fsl = slice(fh * c.VF, (fh + 1) * c.VF)
                    wm = wmp.tile([128, c.GB, c.VF], BF16, tag=f"wm{fh}",
                                  name=f"wm{fh}")
                    nc.scalar.dma_start(
                        out=wm[:],
                        in_=w_mproj[:, fsl].rearrange(
                            "(g p) f -> p g f", p=128))
                    wm_all.append(wm)
                for tb in range(c.TB):
                    yo = yop.tile([128, c.D], F32, tag="yo")
                    for fh in range(c.FH):
                        fsl = slice(fh * c.VF, (fh + 1) * c.VF)
                        ps = psm.tile([128, c.VF], F32, tag="ps")
                        for g in range(c.GB):
                            nc.tensor.matmul(
                                ps[:], gt[g][:, tb * 128:(tb + 1) * 128],
                                wm_all[fh][:, g, :],
                                start=(g == 0), stop=(g == c.GB - 1))
                        nc.vector.tensor_add(yo[:, fsl], ps[:],
                                             x2t[tb][:, fsl])
                        nc.vector.tensor_add(yo[:, fsl], yo[:, fsl],
                                             bmp_b[:, fsl])
                    nc.sync.dma_start(
                        out=y_out[tb * 128:(tb + 1) * 128, :], in_=yo[:])

        es_x2.close()

    nc.compile()
    return nc


def make_core_inputs(cfg: Cfg, x, ln1_w, ln1_b, W_attn, b_attn, W_cproj,
                     b_cproj, ln2_w, ln2_b, W_fc, b_fc, W_mproj, b_mproj):
    """Split full inputs into one in_map per core."""
    c = cfg
    f32 = np.float32
    shared = {
        "w_attn": np.ascontiguousarray(W_attn, f32),
        "w_cproj": np.ascontiguousarray(W_cproj, f32),
        "w_fc": np.ascontiguousarray(W_fc, f32),
        "w_mproj": np.ascontiguousarray(W_mproj).astype(ml_dtypes.bfloat16),
        "ln1w": np.ascontiguousarray(ln1_w, f32).reshape(1, c.D),
        "ln1b": np.ascontiguousarray(ln1_b, f32).reshape(1, c.D),
        "ln2w": np.ascontiguousarray(ln2_w, f32).reshape(1, c.D),
        "ln2b": np.ascontiguousarray(ln2_b, f32).reshape(1, c.D),
        "bv": np.ascontiguousarray(b_attn[2 * c.D:3 * c.D], f32).reshape(1, c.D),
        "bcp": np.ascontiguousarray(b_cproj, f32).reshape(1, c.D),
        "bmp": np.ascontiguousarray(b_mproj, f32).reshape(1, c.D),
        "bfc": np.ascontiguousarray(
            b_fc.astype(f32).reshape(c.GB, 128).T),
        "qidx": np.arange(c.T, dtype=f32).reshape(1, c.T),
    }
    bqk = b_attn[:2 * c.D].astype(f32).reshape(2 * c.DC, 128).T.copy()
    bqk[:, :c.DC] *= 1.0 / math.sqrt(c.HD)
    shared["battn_qk"] = np.ascontiguousarray(bqk)

    del shared["qidx"]
    in_maps = []
    for core in range(c.n_cores):
        b, half = core // 2, core % 2
        rows = core_rows(c, half)
        m = dict(shared)
        m["x"] = np.ascontiguousarray(x[b][rows], f32)
        m["qidx"] = rows.astype(f32).reshape(1, c.T)
        kofs = np.empty((128, c.KC), f32)
        for kc in range(c.KC):
            parity = kc // c.KCH
            loc = (kc % c.KCH) * 128 + np.arange(128)
            kofs[:, kc] = (2 * (loc // c.BS) + parity) * c.BS + loc % c.BS
        m["kofs"] = kofs
        in_maps.append(m)
    return in_maps


def core_rows(cfg, half):
    """absolute sequence rows owned by a core with parity half"""
    c = cfg
    loc = np.arange(c.T)
    return (2 * (loc // c.BS) + half) * c.BS + loc % c.BS


_NC_CACHE = {}


def get_nc(cfg: Cfg):
    key = (cfg.B, cfg.S, cfg.D, cfg.H, cfg.F)
    if key not in _NC_CACHE:
        _NC_CACHE[key] = build(cfg)
    return _NC_CACHE[key]


def kernel(**inputs) -> np.ndarray:
    from concourse.bass_utils import run_bass_kernel_spmd

    cfg = Cfg()
    nc = get_nc(cfg)
    in_maps = make_core_inputs(cfg, **inputs)
    res = run_bass_kernel_spmd(nc, in_maps, core_ids=list(range(cfg.n_cores)))
    B, S, D, T = cfg.B, cfg.S, cfg.D, cfg.T
    out = np.empty((B, S, D), np.float32)
    for core in range(cfg.n_cores):
        b, half = core // 2, core % 2
        out[b, core_rows(cfg, half), :] = res.results[core]["y"]
    return out



# revision 14
# speedup vs baseline: 1.4563x; 1.4563x over previous
"""Single transformer block on 8 NeuronCores — collective-free.

Sharding: core c = (batch b=c//2, parity p=c%2). Each core receives the FULL
sequence of its batch, permuted to [own-stripe | peer-stripe] order, and
recomputes K and V for all 2048 tokens locally — cheaper than the pairwise
AllGather it replaces (~55us extra PE vs ~270us of collective time) and it
deletes all DRAM bounce traffic.  Q / attention / c_proj / MLP cover only the
core's 1024 own (striped) tokens.

Tricks:
  - LayerNorm affine (w, b) folds host-side into the following matmul
    weights/bias, so on-chip LN is just (x - mean) * rsqrt(var + eps).
  - The 1/sqrt(hd) query scale folds host-side into W_q / b_q.
  - V is built directly in [128 key, KC, H, 65] layout with a ones column at
    65, so AV yields the softmax denominator for free and per-head V slices
    are zero-copy views.
  - Scores are computed transposed S^T[k, q]; causal mask is a 0/1 multiply
    on P = exp(S) (finite, exact).
  - AV accumulates a whole 512-query group into one [65, 512] PSUM bank:
    chunks common to both 256-slots run 512-wide, diagonal-extra chunks run
    256-wide into the upper half.
  - Optional fp8 (e4m3) QKV projection with DoubleRow matmuls (2x PE rate,
    half the instructions); weights are pre-scaled x64 host-side so 0.02-std
    values stay out of the fp8 subnormal range, undone at eviction.
"""

import math
from contextlib import ExitStack

import numpy as np
import ml_dtypes

import concourse.bacc as bacc
import concourse.bass as bass
import concourse.mybir as mybir
import concourse.tile as tile
from concourse.masks import make_identity

F32 = mybir.dt.float32
F32R = mybir.dt.float32r
BF16 = mybir.dt.bfloat16
F8 = mybir.dt.float8e4
AF = mybir.ActivationFunctionType
ALU = mybir.AluOpType

EPS = 1e-5


class Cfg:
    def __init__(self, B=4, S=2048, D=1024, H=16, F=4096, n_cores=8,
                 qkv_fp8=False):
        self.B, self.S, self.D, self.H, self.F = B, S, D, H, F
        self.n_cores = n_cores
        assert n_cores == 2 * B
        self.HD = D // H
        assert self.HD == 64
        self.T = S // 2            # tokens owned per core
        self.KT = S // 128         # token 128-tiles, full sequence
        self.TB = self.T // 128    # token 128-tiles, local
        self.DC = D // 128         # contraction chunks over D
        self.QF = 512              # free-dim tile for projection matmuls
        self.KC = S // 128         # key 128-chunks over full sequence
        self.GB = F // 128         # MLP hidden 128-blocks
        self.HPB = 128 // self.HD  # heads per 128-feature block (=2)
        self.BS = 256              # stripe block (q-slot) size
        self.SLOTS = self.T // self.BS
        self.KCH = self.KC // 2    # chunks per parity half
        self.qkv_fp8 = qkv_fp8
        self.wscale = 64.0 if qkv_fp8 else 1.0


def chunk_absblk(c, kc):
    # conservative absolute stripe-block index covered by key chunk kc
    parity = kc // c.KCH
    loc = kc % c.KCH
    return 2 * (loc // (c.BS // 128)) + parity


def build(cfg: Cfg):
    c = cfg
    QT = F8 if c.qkv_fp8 else BF16
    nc = bacc.Bacc(None, target_bir_lowering=False)

    # ---------------- I/O ----------------
    x_in = nc.dram_tensor("x", [c.S, c.D], F32, kind="ExternalInput")
    w_attn = nc.dram_tensor("w_attn", [c.D, 3 * c.D], QT, kind="ExternalInput")
    w_cproj = nc.dram_tensor("w_cproj", [c.D, c.D], BF16, kind="ExternalInput")
    w_fc = nc.dram_tensor("w_fc", [c.D, c.F], BF16, kind="ExternalInput")
    w_mproj = nc.dram_tensor("w_mproj", [c.F, c.D], BF16, kind="ExternalInput")
    battn_qk_in = nc.dram_tensor("battn_qk", [128, 2 * c.DC], F32,
                                 kind="ExternalInput")
    bv_in = nc.dram_tensor("bv", [1, c.D], F32, kind="ExternalInput")
    bcp_in = nc.dram_tensor("bcp", [1, c.D], F32, kind="ExternalInput")
    bmp_in = nc.dram_tensor("bmp", [1, c.D], F32, kind="ExternalInput")
    bfc_in = nc.dram_tensor("bfc", [128, c.GB], F32, kind="ExternalInput")
    qidx_in = nc.dram_tensor("qidx", [1, c.T], F32, kind="ExternalInput")
    kofs_in = nc.dram_tensor("kofs", [128, c.KC], F32, kind="ExternalInput")
    y_out = nc.dram_tensor("y", [c.T, c.D], F32, kind="ExternalOutput")

    def bcast(dram, p=128):
        # partition-broadcast DMA source: read row 0 for every partition
        return bass.AP(tensor=dram, offset=0, ap=[[0, p], [1, dram.shape[1]]])

    with tile.TileContext(nc) as tc, ExitStack() as es:
        gconst = es.enter_context(tc.tile_pool(name="gconst", bufs=1))
        ident = gconst.tile([128, 128], F32)
        make_identity(nc, ident[:])
        eps_t = gconst.tile([128, 1], F32)
        nc.vector.memset(eps_t[:], EPS)
        ones64_f = gconst.tile([1, 64], F32)
        nc.vector.memset(ones64_f[:], 1.0)
        ones64 = gconst.tile([1, 64], F32R)
        nc.vector.tensor_copy(ones64[:], ones64_f[:])

        def layernorm_to(get_src, n_tiles, dest, lnp, ps_tr, tag,
                         interleave=None):
            """normalize token tiles and write feature-major into dest
            [128, DC, n_tiles*128].  get_src(tb) -> token-major [128, D] tile.
            interleave(g) is called after every 4th tile to emit consumer
            work early (keeps PE fed in emission order)."""
            for tb in range(n_tiles):
                src = get_src(tb)
                st = lnp.tile([128, 2, 6], F32, tag=f"{tag}st")
                for sg in range(2):
                    nc.vector.bn_stats(
                        out=st[:, sg, :], in_=src[:, sg * 512:(sg + 1) * 512])
                mv = lnp.tile([128, 2], F32, tag=f"{tag}mv")
                nc.vector.bn_aggr(out=mv[:], in_=st[:])
                sd = lnp.tile([128, 1], F32, tag=f"{tag}sd")
                nc.scalar.activation(sd[:], mv[:, 1:2], AF.Sqrt,
                                     bias=eps_t[:, 0:1])
                rs = lnp.tile([128, 1], F32, tag=f"{tag}rs")
                nc.vector.reciprocal(rs[:], sd[:])
                nrm = lnp.tile([128, c.D], F32, tag=f"{tag}n")
                nc.vector.tensor_scalar(
                    out=nrm[:], in0=src[:], scalar1=mv[:, 0:1],
                    scalar2=rs[:, 0:1], op0=ALU.subtract, op1=ALU.mult)
                for i2 in range(c.DC // 4):
                    pt = ps_tr.tile([128, 512], F32, tag=f"{tag}tr")
                    for j in range(4):
                        ch = 4 * i2 + j
                        nc.tensor.matmul(
                            pt[:, j * 128:(j + 1) * 128],
                            nrm[:, ch * 128:(ch + 1) * 128], ident[:],
                            is_transpose=True, start=(j == 0), stop=(j == 3))
                    nc.scalar.activation(
                        dest[:, 4 * i2:4 * i2 + 4, tb * 128:(tb + 1) * 128],
                        pt[:], AF.Identity)
                if interleave is not None and tb % 4 == 3:
                    interleave(tb // 4)

        # ---------------- persistent activations ----------------
        es_per = ExitStack()
        xloc = []
        xlp = es_per.enter_context(tc.tile_pool(name="xloc", bufs=1,
                                                side="left"))
        for tb in range(c.TB):
            t = xlp.tile([128, c.D], F32, tag=f"x{tb}", name=f"x{tb}")
            nc.sync.dma_start(out=t[:], in_=x_in[tb * 128:(tb + 1) * 128, :])
            xloc.append(t)

        es_kvq = ExitStack()
        kvqp = es_kvq.enter_context(tc.tile_pool(name="kvq", bufs=1,
                                                 side="right"))
        ktp = kvqp.tile([128, c.DC, c.S], BF16, name="ktp")
        vtt = kvqp.tile([128, c.KC, c.H, 65], BF16, name="vtt")
        qtp = kvqp.tile([128, c.DC, c.T], BF16, name="qtp")
        nc.vector.memset(vtt[:, :, :, 64:65], 1.0)

        # ================= phase A: LN1 + QKV =================
        es_ht = ExitStack()
        htp = es_ht.enter_context(tc.tile_pool(name="htp", bufs=1))
        ht = htp.tile([128, c.DC, c.S], QT, name="ht")

        with (
            tc.tile_pool(name="aconst", bufs=1) as aconst,
            tc.tile_pool(name="xs", bufs=2) as xsp,
            tc.tile_pool(name="lnp", bufs=2) as lnp,
            tc.tile_pool(name="wa", bufs=2) as wap,
            tc.tile_pool(name="ko", bufs=4) as kop,
            tc.tile_pool(name="ps_tr", bufs=3, space="PSUM") as ps_tr,
            tc.tile_pool(name="ps_mm", bufs=4, space="PSUM") as psmm,
        ):
            battn_qk = aconst.tile([128, 2 * c.DC], F32)
            nc.sync.dma_start(out=battn_qk[:], in_=battn_qk_in[:, :])
            bv_b = aconst.tile([128, c.D], F32)
            nc.sync.dma_start(out=bv_b[:], in_=bcast(bv_in))
            bcp_b = aconst.tile([128, c.D], F32)
            nc.sync.dma_start(out=bcp_b[:], in_=bcast(bcp_in))

            wk = wap.tile([128, c.DC, c.D], QT, tag="wa", name="wk")
            nc.scalar.dma_start(
                out=wk[:],
                in_=w_attn[:, c.D:2 * c.D].rearrange("(i p) f -> p i f",
                                                     p=128))
            wv = wap.tile([128, c.DC, c.D], QT, tag="wa", name="wv")
            nc.scalar.dma_start(
                out=wv[:],
                in_=w_attn[:, 2 * c.D:3 * c.D].rearrange("(i p) f -> p i f",
                                                         p=128))

            inv_w = 1.0 / c.wscale

            def proj_acc(ps, wslab, msl, rhs_tok_slice):
                """accumulate psum[:, :] = sum_i W[:,i,msl].T @ ht[:,i,toks]"""
                if c.qkv_fp8:
                    for i2 in range(c.DC // 2):
                        nc.tensor.matmul(
                            ps, wslab[:, 2 * i2:2 * i2 + 2, msl],
                            ht[:, 2 * i2:2 * i2 + 2, rhs_tok_slice],
                            start=(i2 == 0), stop=(i2 == c.DC // 2 - 1),
                            perf_mode=mybir.MatmulPerfMode.DoubleRow)
                else:
                    for i in range(c.DC):
                        nc.tensor.matmul(
                            ps, wslab[:, i, msl],
                            ht[:, i, rhs_tok_slice],
                            start=(i == 0), stop=(i == c.DC - 1))

            def kv_chunk(g):
                tsl = slice(g * 512, (g + 1) * 512)
                # K^T for this 512-token slice, all feature chunks
                for m in range(c.DC):
                    ps = psmm.tile([128, 512], F32, tag="ps")
                    proj_acc(ps[:], wk, slice(m * 128, (m + 1) * 128), tsl)
                    nc.scalar.activation(
                        ktp[:, m, tsl], ps[:], AF.Identity,
                        bias=battn_qk[:, c.DC + m:c.DC + m + 1], scale=inv_w)
                # V for these 4 token tiles, both feature halves
                for tb in range(4 * g, 4 * g + 4):
                    for vh in range(2):
                        fsl = slice(vh * 512, (vh + 1) * 512)
                        ps = psmm.tile([128, 512], F32, tag="ps")
                        if c.qkv_fp8:
                            for i2 in range(c.DC // 2):
                                nc.tensor.matmul(
                                    ps[:],
                                    ht[:, 2 * i2:2 * i2 + 2,
                                       tb * 128:(tb + 1) * 128],
                                    wv[:, 2 * i2:2 * i2 + 2, fsl],
                                    start=(i2 == 0),
                                    stop=(i2 == c.DC // 2 - 1),
                                    perf_mode=mybir.MatmulPerfMode.DoubleRow)
                        else:
                            for i in range(c.DC):
                                nc.tensor.matmul(
                                    ps[:], ht[:, i, tb * 128:(tb + 1) * 128],
                                    wv[:, i, fsl],
                                    start=(i == 0), stop=(i == c.DC - 1))
                        dst = vtt[:, tb, vh * 8:(vh + 1) * 8, 0:64]
                        if c.qkv_fp8:
                            vo = kop.tile([128, 512], BF16, tag="vo")
                            nc.scalar.activation(vo[:], ps[:], AF.Identity,
                                                 scale=inv_w)
                            nc.vector.tensor_add(dst, vo[:], bv_b[:, fsl])
                        else:
                            nc.vector.tensor_add(dst, ps[:], bv_b[:, fsl])

            def get_src(tb):
                if tb < c.TB:
                    return xloc[tb]
                t = xsp.tile([128, c.D], F32, tag="xs")
                nc.sync.dma_start(out=t[:],
                                  in_=x_in[tb * 128:(tb + 1) * 128, :])
                return t

            layernorm_to(get_src, c.KT, ht, lnp, ps_tr, "a",
                         interleave=kv_chunk)

            # ---- q^T pass (local tokens only; scale folded host-side) ----
            wq = wap.tile([128, c.DC, c.D], QT, tag="wa", name="wq")
            nc.scalar.dma_start(
                out=wq[:],
                in_=w_attn[:, 0:c.D].rearrange("(i p) f -> p i f", p=128))
            for m in range(c.DC):
                for th in range(c.T // 512):
                    tsl = slice(th * 512, (th + 1) * 512)
                    ps = psmm.tile([128, 512], F32, tag="ps")
                    proj_acc(ps[:], wq, slice(m * 128, (m + 1) * 128), tsl)
                    nc.scalar.activation(
                        qtp[:, m, tsl], ps[:], AF.Identity,
                        bias=battn_qk[:, m:m + 1], scale=inv_w)

            # fold the c_proj bias into the residual copy of x, in place
            for tb in range(c.TB):
                nc.vector.tensor_add(xloc[tb][:], xloc[tb][:], bcp_b[:])

        es_ht.close()

        # ================= phase B: attention =================
        # prefetch c_proj weights during attention
        es_wc = ExitStack()
        wcp = es_wc.enter_context(tc.tile_pool(name="wc", bufs=1,
                                               side="left"))
        wc = wcp.tile([128, c.DC, c.D], BF16, name="wc")
        nc.scalar.dma_start(
            out=wc[:], in_=w_cproj[:, :].rearrange("(i p) f -> p i f", p=128))

        es_at = ExitStack()
        atp = es_at.enter_context(tc.tile_pool(name="atp", bufs=1,
                                               side="left"))
        at = atp.tile([128, c.DC, c.T], BF16, name="at")

        with (
            tc.tile_pool(name="bconst", bufs=1) as bconst,
            tc.tile_pool(name="mask", bufs=1) as maskp,
            tc.tile_pool(name="pt", bufs=3) as ptp,
            tc.tile_pool(name="rec", bufs=3) as recp,
            tc.tile_pool(name="ps_qk", bufs=2, space="PSUM") as psqk,
            tc.tile_pool(name="ps_od", bufs=1, space="PSUM") as psod,
            tc.tile_pool(name="ps_o", bufs=2, space="PSUM") as pso,
            tc.tile_pool(name="ps_bc", bufs=1, space="PSUM") as psbc,
        ):
            qidx = bconst.tile([128, c.T], F32)
            nc.sync.dma_start(out=qidx[:], in_=bcast(qidx_in))
            kofs = bconst.tile([128, c.KC], F32)
            nc.sync.dma_start(out=kofs[:], in_=kofs_in[:, :])

            # per-slot chunk lists (compile-time causal structure)
            slot_chunks = []
            for sl in range(c.SLOTS):
                cl = [kc for kc in range(c.KC)
                      if chunk_absblk(c, kc) <= 2 * sl + 1]
                slot_chunks.append(cl)
            groups = [[0, 1], [2, 3]]

            # boundary masks per (group, chunk) that may cross a diagonal
            masks = {}
            for gi, g in enumerate(groups):
                gw = len(g) * c.BS
                qsl = slice(g[0] * c.BS, g[0] * c.BS + gw)
                for kc in slot_chunks[g[-1]]:
                    if chunk_absblk(c, kc) >= 2 * g[0]:
                        mk = maskp.tile([128, gw], BF16, tag=f"mk{gi}_{kc}",
                                        name=f"mk{gi}_{kc}")
                        nc.vector.tensor_scalar(
                            out=mk[:], in0=qidx[:, qsl],
                            scalar1=kofs[:, kc:kc + 1], scalar2=None,
                            op0=ALU.is_ge)
                        masks[(gi, kc)] = mk

            def pairs(lst):
                return [lst[i:i + 2] for i in range(0, len(lst), 2)]

            for jj in range(c.DC):
                for hp in range(c.HPB):
                    h = c.HPB * jj + hp
                    base = hp * 64
                    for gi, g in enumerate(groups):
                        gq = slice(g[0] * c.BS, g[0] * c.BS + 512)
                        uq = slice(g[1] * c.BS, (g[1] + 1) * c.BS)
                        cl = slot_chunks[g[-1]]
                        common = [kc for kc in cl
                                  if chunk_absblk(c, kc) <= 2 * g[0] + 1]
                        diff = [kc for kc in cl
                                if chunk_absblk(c, kc) > 2 * g[0] + 1]
                        po = pso.tile([65, 512], F32, tag="po")
                        n_av = len(common) + len(diff)
                        n = 0
                        for pr in pairs(common):
                            ps2 = psqk.tile([128, 1024], F32, tag="qk")
                            pt = ptp.tile([128, 1024], BF16, tag="pt")
                            for ix, kc in enumerate(pr):
                                nc.tensor.matmul(
                                    ps2[:, ix * 512:(ix + 1) * 512],
                                    ktp[base:base + 64, jj,
                                        kc * 128:(kc + 1) * 128],
                                    qtp[base:base + 64, jj, gq],
                                    start=True, stop=True)
                            w = len(pr) * 512
                            nc.scalar.activation(pt[:, 0:w], ps2[:, 0:w],
                                                 AF.Exp)
                            for ix, kc in enumerate(pr):
                                if (gi, kc) in masks:
                                    psl = slice(ix * 512, ix * 512 + 512)
                                    nc.vector.tensor_mul(
                                        pt[:, psl], pt[:, psl],
                                        masks[(gi, kc)][:])
                            for ix, kc in enumerate(pr):
                                nc.tensor.matmul(
                                    po[:], vtt[:, kc, h, :],
                                    pt[:, ix * 512:(ix + 1) * 512],
                                    start=(n == 0), stop=(n == n_av - 1))
                                n += 1
                        for pr in pairs(diff):
                            ps1 = psod.tile([128, 512], F32, tag="od")
                            pt = ptp.tile([128, 1024], BF16, tag="pt")
                            for ix, kc in enumerate(pr):
                                nc.tensor.matmul(
                                    ps1[:, ix * 256:(ix + 1) * 256],
                                    ktp[base:base + 64, jj,
                                        kc * 128:(kc + 1) * 128],
                                    qtp[base:base + 64, jj, uq],
                                    start=True, stop=True)
                            w = len(pr) * 256
                            nc.scalar.activation(pt[:, 0:w], ps1[:, 0:w],
                                                 AF.Exp)
                            for ix, kc in enumerate(pr):
                                if (gi, kc) in masks:
                                    psl = slice(ix * 256, ix * 256 + 256)
                                    nc.vector.tensor_mul(
                                        pt[:, psl], pt[:, psl],
                                        masks[(gi, kc)][:, c.BS:2 * c.BS])
                            for ix, kc in enumerate(pr):
                                nc.tensor.matmul(
                                    po[:, 256:512], vtt[:, kc, h, :],
                                    pt[:, ix * 256:(ix + 1) * 256],
                                    start=(n == 0), stop=(n == n_av - 1))
                                n += 1
                        # normalize by softmax denominator (row 64)
                        rec = recp.tile([1, 512], F32R, tag="rec")
                        with nc.allow_low_precision(
                                reason="softmax denom in f32r"):
                            nc.vector.reciprocal(rec[:], po[64:65, :])
                        bc = psbc.tile([64, 512], F32, tag="bc")
                        nc.tensor.matmul(bc[:], ones64[:], rec[:],
                                         start=True, stop=True)
                        bcs = recp.tile([64, 512], F32, tag="bcs")
                        nc.vector.tensor_copy(bcs[:], bc[:])
                        nc.vector.tensor_mul(
                            at[base:base + 64, jj, gq], po[0:64, :], bcs[:])

        es_kvq.close()

        # ================= phase C: c_proj + residual =================
        es_x2 = ExitStack()
        x2p = es_x2.enter_context(tc.tile_pool(name="x2p", bufs=1,
                                               side="right"))
        x2t = []
        with tc.tile_pool(name="ps_c", bufs=4, space="PSUM") as psc:
            for tb in range(c.TB):
                x2 = x2p.tile([128, c.D], F32, tag=f"x2_{tb}",
                              name=f"x2_{tb}")
                for fh in range(2):
                    fsl = slice(fh * 512, (fh + 1) * 512)
                    ps = psc.tile([128, 512], F32, tag="ps")
                    for i in range(c.DC):
                        nc.tensor.matmul(
                            ps[:], at[:, i, tb * 128:(tb + 1) * 128],
                            wc[:, i, fsl],
                            start=(i == 0), stop=(i == c.DC - 1))
                    nc.vector.tensor_add(x2[:, fsl], ps[:],
                                         xloc[tb][:, fsl])
                x2t.append(x2)

        es_at.close()
        es_wc.close()
        es_per.close()

        # ================= phase D: LN2 + MLP =================
        with (
            tc.tile_pool(name="dconst", bufs=1) as dconst,
            tc.tile_pool(name="gt", bufs=1) as gtp,
            tc.tile_pool(name="wm", bufs=1) as wmp,
        ):
            bmp_b = dconst.tile([128, c.D], F32)
            nc.sync.dma_start(out=bmp_b[:], in_=bcast(bmp_in))
            bfc = dconst.tile([128, c.GB], F32)
            nc.sync.dma_start(out=bfc[:], in_=bfc_in[:, :])

            gt = gtp.tile([128, c.GB, c.T], BF16, name="gt")
            # mproj weights prefetched during LN2/fc on the sync queue so
            # they don't serialize behind the fc slab loads (scalar queue)
            wm_all = []
            for fh in range(2):
                wm = wmp.tile([128, c.GB, 512], BF16, tag=f"wm{fh}",
                              name=f"wm{fh}")
                nc.sync.dma_start(
                    out=wm[:],
                    in_=w_mproj[:, fh * 512:(fh + 1) * 512].rearrange(
                        "(g p) f -> p g f", p=128))
                wm_all.append(wm)
            with (
                tc.tile_pool(name="mtp", bufs=1) as mtp,
                tc.tile_pool(name="lnp2", bufs=2) as lnp2,
                tc.tile_pool(name="wf", bufs=2) as wfp,
                tc.tile_pool(name="ps_tr2", bufs=3, space="PSUM") as ps_tr2,
                tc.tile_pool(name="ps_g", bufs=2, space="PSUM") as psg,
            ):
                mt = mtp.tile([128, c.DC, c.T], BF16, name="mt")
                layernorm_to(lambda tb: x2t[tb], c.TB, mt, lnp2, ps_tr2, "d")

                # ---------------- fc + gelu ----------------
                wf = None
                for gb in range(c.GB):
                    if gb % 4 == 0:
                        wf = wfp.tile([128, c.DC, 512], BF16, tag="wf",
                                      name=f"wf{gb}")
                        j = gb // 4
                        nc.scalar.dma_start(
                            out=wf[:],
                            in_=w_fc[:, j * 512:(j + 1) * 512].rearrange(
                                "(i p) f -> p i f", p=128))
                    gl = (gb % 4) * 128
                    ps = psg.tile([128, 1024], F32, tag="ps")
                    for th in range(2):
                        for i in range(c.DC):
                            nc.tensor.matmul(
                                ps[:, th * 512:(th + 1) * 512],
                                wf[:, i, gl:gl + 128],
                                mt[:, i, th * 512:(th + 1) * 512],
                                start=(i == 0), stop=(i == c.DC - 1))
                    nc.scalar.activation(
                        gt[:, gb, :], ps[:], AF.Gelu_apprx_tanh,
                        bias=bfc[:, gb:gb + 1])

            # ---------------- mproj + residual ----------------
            with (
                tc.tile_pool(name="yout", bufs=3) as yop,
                tc.tile_pool(name="ps_m", bufs=4, space="PSUM") as psm,
            ):
                for tb in range(c.TB):
                    yo = yop.tile([128, c.D], F32, tag="yo")
                    for fh in range(2):
                        fsl = slice(fh * 512, (fh + 1) * 512)
                        ps = psm.tile([128, 512], F32, tag="ps")
                        for g in range(c.GB):
                            nc.tensor.matmul(
                                ps[:], gt[:, g, tb * 128:(tb + 1) * 128],
                                wm_all[fh][:, g, :],
                                start=(g == 0), stop=(g == c.GB - 1))
                        nc.vector.tensor_add(yo[:, fsl], ps[:],
                                             x2t[tb][:, fsl])
                        nc.vector.tensor_add(yo[:, fsl], yo[:, fsl],
                                             bmp_b[:, fsl])
                    nc.sync.dma_start(
                        out=y_out[tb * 128:(tb + 1) * 128, :], in_=yo[:])

        es_x2.close()

    nc.compile()
    return nc


def core_rows(cfg, half):
    """absolute sequence rows owned by a core with parity half"""
    c = cfg
    loc = np.arange(c.T)
    return (2 * (loc // c.BS) + half) * c.BS + loc % c.BS


def make_core_inputs(cfg: Cfg, x, ln1_w, ln1_b, W_attn, b_attn, W_cproj,
                     b_cproj, ln2_w, ln2_b, W_fc, b_fc, W_mproj, b_mproj):
    """Split full inputs into one in_map per core."""
    c = cfg
    f32 = np.float32
    qt = ml_dtypes.float8_e4m3fn if c.qkv_fp8 else ml_dtypes.bfloat16

    # fold LN1 affine + query scale into W_attn / b_attn
    ln1_w = np.asarray(ln1_w, f32)
    ln1_b = np.asarray(ln1_b, f32)
    Wa = np.asarray(W_attn, f32) * ln1_w[:, None]
    ba = np.asarray(b_attn, f32) + ln1_b @ np.asarray(W_attn, f32)
    qs = 1.0 / math.sqrt(c.HD)
    Wa = Wa.copy()
    Wa[:, :c.D] *= qs
    ba = ba.copy()
    ba[:c.D] *= qs
    Wa_dev = (Wa * c.wscale).astype(qt)

    # fold LN2 affine into W_fc / b_fc
    ln2_w = np.asarray(ln2_w, f32)
    ln2_b = np.asarray(ln2_b, f32)
    Wf = np.asarray(W_fc, f32) * ln2_w[:, None]
    bf = np.asarray(b_fc, f32) + ln2_b @ np.asarray(W_fc, f32)

    shared = {
        "w_attn": np.ascontiguousarray(Wa_dev),
        "w_cproj": np.ascontiguousarray(W_cproj).astype(ml_dtypes.bfloat16),
        "w_fc": np.ascontiguousarray(Wf).astype(ml_dtypes.bfloat16),
        "w_mproj": np.ascontiguousarray(W_mproj).astype(ml_dtypes.bfloat16),
        "bv": np.ascontiguousarray(ba[2 * c.D:3 * c.D]).reshape(1, c.D),
        "bcp": np.ascontiguousarray(b_cproj, f32).reshape(1, c.D),
        "bmp": np.ascontiguousarray(b_mproj, f32).reshape(1, c.D),
        "bfc": np.ascontiguousarray(bf.reshape(c.GB, 128).T),
        "battn_qk": np.ascontiguousarray(
            ba[:2 * c.D].reshape(2 * c.DC, 128).T),
    }

    x = np.asarray(x, f32)
    in_maps = []
    for core in range(c.n_cores):
        b, half = core // 2, core % 2
        own = core_rows(c, half)
        peer = core_rows(c, 1 - half)
        perm = np.concatenate([own, peer])
        m = dict(shared)
        m["x"] = np.ascontiguousarray(x[b][perm])
        m["qidx"] = own.astype(f32).reshape(1, c.T)
        kofs = np.empty((128, c.KC), f32)
        for kc in range(c.KC):
            kofs[:, kc] = perm[kc * 128 + np.arange(128)]
        m["kofs"] = kofs
        in_maps.append(m)
    return in_maps


_NC_CACHE = {}


def get_nc(cfg: Cfg):
    key = (cfg.B, cfg.S, cfg.D, cfg.H, cfg.F, cfg.qkv_fp8)
    if key not in _NC_CACHE:
        _NC_CACHE[key] = build(cfg)
    return _NC_CACHE[key]


def kernel(**inputs) -> np.ndarray:
    from concourse.bass_utils import run_bass_kernel_spmd

    cfg = Cfg()
    nc = get_nc(cfg)
    in_maps = make_core_inputs(cfg, **inputs)
    res = run_bass_kernel_spmd(nc, in_maps, core_ids=list(range(cfg.n_cores)))
    B, S, D = cfg.B, cfg.S, cfg.D
    out = np.empty((B, S, D), np.float32)
    for core in range(cfg.n_cores):
        b, half = core // 2, core % 2
        out[b, core_rows(cfg, half), :] = res.results[core]["y"]
    return out


# revision 27
# speedup vs baseline: 1.8628x; 1.2791x over previous
"""Single transformer block on 8 NeuronCores — collective-free.

Sharding: core c = (batch b=c//2, parity p=c%2). Each core receives the FULL
sequence of its batch, permuted to [own-stripe | peer-stripe] order, and
recomputes K and V for all 2048 tokens locally — cheaper than the pairwise
AllGather it replaces (~55us extra PE vs ~270us of collective time) and it
deletes all DRAM bounce traffic.  Q / attention / c_proj / MLP cover only the
core's 1024 own (striped) tokens.

Tricks:
  - LayerNorm affine (w, b) folds host-side into the following matmul
    weights/bias, so on-chip LN is just (x - mean) * rsqrt(var + eps).
  - The 1/sqrt(hd) query scale folds host-side into W_q / b_q.
  - V is built directly in [128 key, KC, H, 65] layout with a ones column at
    65, so AV yields the softmax denominator for free and per-head V slices
    are zero-copy views.
  - Scores are computed transposed S^T[k, q]; causal mask is a 0/1 multiply
    on P = exp(S) (finite, exact).
  - AV accumulates a whole 512-query group into one [65, 512] PSUM bank:
    chunks common to both 256-slots run 512-wide, diagonal-extra chunks run
    256-wide into the upper half.
  - Optional fp8 (e4m3) QKV projection with DoubleRow matmuls (2x PE rate,
    half the instructions); weights are pre-scaled x64 host-side so 0.02-std
    values stay out of the fp8 subnormal range, undone at eviction.
"""

import math
from contextlib import ExitStack

import numpy as np
import ml_dtypes

import concourse.bacc as bacc
import concourse.bass as bass
import concourse.mybir as mybir
import concourse.tile as tile
from concourse.masks import make_identity

F32 = mybir.dt.float32
F32R = mybir.dt.float32r
BF16 = mybir.dt.bfloat16
F8 = mybir.dt.float8e4
AF = mybir.ActivationFunctionType
ALU = mybir.AluOpType

EPS = 1e-5


class Cfg:
    def __init__(self, B=4, S=2048, D=1024, H=16, F=4096, n_cores=8,
                 qkv_fp8=True, fc_fp8=False, mproj_fp8=True, av_fp8=True,
                 BS=128):
        self.B, self.S, self.D, self.H, self.F = B, S, D, H, F
        self.n_cores = n_cores
        assert n_cores == 2 * B
        self.HD = D // H
        assert self.HD == 64
        self.T = S // 2            # tokens owned per core
        self.KT = S // 128         # token 128-tiles, full sequence
        self.TB = self.T // 128    # token 128-tiles, local
        self.DC = D // 128         # contraction chunks over D
        self.QF = 512              # free-dim tile for projection matmuls
        self.KC = S // 128         # key 128-chunks over full sequence
        self.GB = F // 128         # MLP hidden 128-blocks
        self.HPB = 128 // self.HD  # heads per 128-feature block (=2)
        self.BS = BS               # stripe block (q-slot) size
        self.SLOTS = self.T // self.BS
        self.SPG = 512 // self.BS  # slots per 512-wide attention group
        self.KCH = self.KC // 2    # chunks per parity half
        self.CPB = self.BS // 128  # key chunks per stripe block
        self.qkv_fp8 = qkv_fp8
        self.fc_fp8 = fc_fp8
        self.mproj_fp8 = mproj_fp8
        self.av_fp8 = av_fp8
        self.wscale = 64.0 if qkv_fp8 else 1.0
        self.fscale = 64.0 if fc_fp8 else 1.0
        self.mscale = 64.0 if mproj_fp8 else 1.0


def chunk_absblk(c, kc):
    # conservative absolute stripe-block index covered by key chunk kc
    parity = kc // c.KCH
    loc = kc % c.KCH
    return 2 * (loc // (c.BS // 128)) + parity


def build(cfg: Cfg):
    c = cfg
    QT = F8 if c.qkv_fp8 else BF16
    FT = F8 if c.fc_fp8 else BF16
    MPT = F8 if c.mproj_fp8 else BF16
    VT = F8 if c.av_fp8 else BF16
    PT = VT
    nc = bacc.Bacc(None, target_bir_lowering=False)

    # ---------------- I/O ----------------
    x_in = nc.dram_tensor("x", [c.S, c.D], F32, kind="ExternalInput")
    w_attn = nc.dram_tensor("w_attn", [c.D, 3 * c.D], QT, kind="ExternalInput")
    w_cproj = nc.dram_tensor("w_cproj", [c.D, c.D], BF16, kind="ExternalInput")
    w_fc = nc.dram_tensor("w_fc", [c.D, c.F], FT, kind="ExternalInput")
    w_mproj = nc.dram_tensor("w_mproj", [c.F, c.D], MPT,
                             kind="ExternalInput")
    battn_qk_in = nc.dram_tensor("battn_qk", [128, 2 * c.DC], F32,
                                 kind="ExternalInput")
    bv_in = nc.dram_tensor("bv", [1, c.D], F32, kind="ExternalInput")
    bcp_in = nc.dram_tensor("bcp", [1, c.D], F32, kind="ExternalInput")
    bmp_in = nc.dram_tensor("bmp", [1, c.D], F32, kind="ExternalInput")
    bfc_in = nc.dram_tensor("bfc", [128, c.GB], F32, kind="ExternalInput")
    qidx_in = nc.dram_tensor("qidx", [1, c.T], F32, kind="ExternalInput")
    kofs_in = nc.dram_tensor("kofs", [128, c.KC], F32, kind="ExternalInput")
    y_out = nc.dram_tensor("y", [c.T, c.D], F32, kind="ExternalOutput")

    def bcast(dram, p=128):
        # partition-broadcast DMA source: read row 0 for every partition
        return bass.AP(tensor=dram, offset=0, ap=[[0, p], [1, dram.shape[1]]])

    with tile.TileContext(nc) as tc, ExitStack() as es:
        gconst = es.enter_context(tc.tile_pool(name="gconst", bufs=1))
        ident = gconst.tile([128, 128], F32)
        make_identity(nc, ident[:])
        eps_t = gconst.tile([128, 1], F32)
        nc.vector.memset(eps_t[:], EPS)
        ones64_f = gconst.tile([1, 64], F32)
        nc.vector.memset(ones64_f[:], 1.0)
        ones64 = gconst.tile([1, 64], F32R)
        nc.vector.tensor_copy(ones64[:], ones64_f[:])

        def layernorm_to(get_src, n_tiles, dest, lnp, ps_tr, tag,
                         interleave=None):
            """normalize token tiles and write feature-major into dest
            [128, DC, n_tiles*128].  get_src(tb) -> token-major [128, D] tile.
            interleave(g) is called after every 4th tile to emit consumer
            work early (keeps PE fed in emission order)."""
            for tb in range(n_tiles):
                src = get_src(tb)
                st = lnp.tile([128, 2, 6], F32, tag=f"{tag}st")
                for sg in range(2):
                    nc.vector.bn_stats(
                        out=st[:, sg, :], in_=src[:, sg * 512:(sg + 1) * 512])
                mv = lnp.tile([128, 2], F32, tag=f"{tag}mv")
                nc.vector.bn_aggr(out=mv[:], in_=st[:])
                sd = lnp.tile([128, 1], F32, tag=f"{tag}sd")
                nc.scalar.activation(sd[:], mv[:, 1:2], AF.Sqrt,
                                     bias=eps_t[:, 0:1])
                rs = lnp.tile([128, 1], F32, tag=f"{tag}rs")
                nc.vector.reciprocal(rs[:], sd[:])
                nrm = lnp.tile([128, c.D], F32, tag=f"{tag}n")
                nc.vector.tensor_scalar(
                    out=nrm[:], in0=src[:], scalar1=mv[:, 0:1],
                    scalar2=rs[:, 0:1], op0=ALU.subtract, op1=ALU.mult)
                for i2 in range(c.DC // 4):
                    pt = ps_tr.tile([128, 512], F32, tag=f"{tag}tr")
                    for j in range(4):
                        ch = 4 * i2 + j
                        nc.tensor.matmul(
                            pt[:, j * 128:(j + 1) * 128],
                            nrm[:, ch * 128:(ch + 1) * 128], ident[:],
                            is_transpose=True, start=(j == 0), stop=(j == 3))
                    nc.scalar.activation(
                        dest[:, 4 * i2:4 * i2 + 4, tb * 128:(tb + 1) * 128],
                        pt[:], AF.Identity)
                if interleave is not None and tb % 4 == 3:
                    interleave(tb // 4)

        # ---------------- persistent activations ----------------
        es_per = ExitStack()
        xloc = []
        xlp = es_per.enter_context(tc.tile_pool(name="xloc", bufs=1,
                                                side="left"))
        for tb in range(c.TB):
            t = xlp.tile([128, c.D], F32, tag=f"x{tb}", name=f"x{tb}")
            nc.sync.dma_start(out=t[:], in_=x_in[tb * 128:(tb + 1) * 128, :])
            xloc.append(t)

        es_kvq = ExitStack()
        kvqp = es_kvq.enter_context(tc.tile_pool(name="kvq", bufs=1,
                                                 side="right"))
        ktp = kvqp.tile([128, c.DC, c.S], BF16, name="ktp")
        vtt = kvqp.tile([128, 2, c.KCH, c.H, 65], VT, name="vtt")
        qtp = kvqp.tile([128, c.DC, c.T], BF16, name="qtp")
        nc.vector.memset(vtt[:, :, :, :, 64:65], 1.0)

        # ================= phase A: LN1 + QKV =================
        es_ht = ExitStack()
        htp = es_ht.enter_context(tc.tile_pool(name="htp", bufs=1))
        ht = htp.tile([128, c.DC, c.S], QT, name="ht")

        with (
            tc.tile_pool(name="aconst", bufs=1) as aconst,
            tc.tile_pool(name="xs", bufs=2) as xsp,
            tc.tile_pool(name="lnp", bufs=2) as lnp,
            tc.tile_pool(name="wa", bufs=2) as wap,
            tc.tile_pool(name="ko", bufs=4) as kop,
            tc.tile_pool(name="ps_tr", bufs=3, space="PSUM") as ps_tr,
            tc.tile_pool(name="ps_mm", bufs=5, space="PSUM") as psmm,
        ):
            battn_qk = aconst.tile([128, 2 * c.DC], F32)
            nc.sync.dma_start(out=battn_qk[:], in_=battn_qk_in[:, :])
            bv_b = aconst.tile([128, c.D], F32)
            nc.sync.dma_start(out=bv_b[:], in_=bcast(bv_in))
            bcp_b = aconst.tile([128, c.D], F32)
            nc.sync.dma_start(out=bcp_b[:], in_=bcast(bcp_in))

            wk = wap.tile([128, c.DC, c.D], QT, tag="wa", name="wk")
            nc.scalar.dma_start(
                out=wk[:],
                in_=w_attn[:, c.D:2 * c.D].rearrange("(i p) f -> p i f",
                                                     p=128))
            wv = wap.tile([128, c.DC, c.D], QT, tag="wa", name="wv")
            nc.scalar.dma_start(
                out=wv[:],
                in_=w_attn[:, 2 * c.D:3 * c.D].rearrange("(i p) f -> p i f",
                                                         p=128))

            inv_w = 1.0 / c.wscale
            NI = c.DC // 2 if c.qkv_fp8 else c.DC  # contraction steps

            def wsl(wslab, i, fsl):
                # weight slab contraction-step slice (pair of chunks in fp8)
                if c.qkv_fp8:
                    return wslab[:, 2 * i:2 * i + 2, fsl]
                return wslab[:, i, fsl]

            def hsl(i, tsl):
                if c.qkv_fp8:
                    return ht[:, 2 * i:2 * i + 2, tsl]
                return ht[:, i, tsl]

            PM = mybir.MatmulPerfMode.DoubleRow if c.qkv_fp8 else None

            def v_chunk(g):
                # V for these 4 token tiles; lhsT (ht slice) shared across
                # both feature halves so ldweights is loaded once per step
                for tb in range(4 * g, 4 * g + 4):
                    tbs = slice(tb * 128, (tb + 1) * 128)
                    pss = [psmm.tile([128, 512], F32, tag="ps", name=f"psv{q}")
                           for q in range(2)]
                    for i in range(NI):
                        for vh in range(2):
                            nc.tensor.matmul(
                                pss[vh][:], hsl(i, tbs),
                                wsl(wv, i, slice(vh * 512, (vh + 1) * 512)),
                                start=(i == 0), stop=(i == NI - 1),
                                perf_mode=PM)
                    for vh in range(2):
                        fsl = slice(vh * 512, (vh + 1) * 512)
                        dst = vtt[:, tb // c.KCH, tb % c.KCH,
                                  vh * 8:(vh + 1) * 8, 0:64]
                        if c.qkv_fp8:
                            vo = kop.tile([128, 512], BF16, tag="vo")
                            nc.scalar.activation(vo[:], pss[vh][:],
                                                 AF.Identity, scale=inv_w)
                            nc.vector.tensor_add(dst, vo[:], bv_b[:, fsl])
                        else:
                            nc.vector.tensor_add(dst, pss[vh][:],
                                                 bv_b[:, fsl])

            def get_src(tb):
                if tb < c.TB:
                    return xloc[tb]
                t = xsp.tile([128, c.D], F32, tag="xs")
                nc.sync.dma_start(out=t[:],
                                  in_=x_in[tb * 128:(tb + 1) * 128, :])
                return t

            layernorm_to(get_src, c.KT, ht, lnp, ps_tr, "a",
                         interleave=v_chunk)

            # ---- k^T pass: lhsT (w chunk) shared across 4 token slices ----
            for m in range(c.DC):
                msl = slice(m * 128, (m + 1) * 128)
                pss = [psmm.tile([128, 512], F32, tag="ps", name=f"psk{q}")
                       for q in range(4)]
                for i in range(NI):
                    for th in range(4):
                        nc.tensor.matmul(
                            pss[th][:], wsl(wk, i, msl),
                            hsl(i, slice(th * 512, (th + 1) * 512)),
                            start=(i == 0), stop=(i == NI - 1),
                            perf_mode=PM)
                for th in range(4):
                    nc.scalar.activation(
                        ktp[:, m, th * 512:(th + 1) * 512], pss[th][:],
                        AF.Identity,
                        bias=battn_qk[:, c.DC + m:c.DC + m + 1], scale=inv_w)

            # ---- q^T pass (local tokens only; scale folded host-side) ----
            wq = wap.tile([128, c.DC, c.D], QT, tag="wa", name="wq")
            nc.scalar.dma_start(
                out=wq[:],
                in_=w_attn[:, 0:c.D].rearrange("(i p) f -> p i f", p=128))
            for m in range(c.DC):
                msl = slice(m * 128, (m + 1) * 128)
                pss = [psmm.tile([128, 512], F32, tag="ps", name=f"psq{q}")
                       for q in range(2)]
                for i in range(NI):
                    for th in range(2):
                        nc.tensor.matmul(
                            pss[th][:], wsl(wq, i, msl),
                            hsl(i, slice(th * 512, (th + 1) * 512)),
                            start=(i == 0), stop=(i == NI - 1),
                            perf_mode=PM)
                for th in range(2):
                    nc.scalar.activation(
                        qtp[:, m, th * 512:(th + 1) * 512], pss[th][:],
                        AF.Identity, bias=battn_qk[:, m:m + 1], scale=inv_w)

            # fold the c_proj bias into the residual copy of x, in place
            for tb in range(c.TB):
                nc.vector.tensor_add(xloc[tb][:], xloc[tb][:], bcp_b[:])

        es_ht.close()

        # ================= phase B: attention =================
        # prefetch c_proj weights during attention
        es_wc = ExitStack()
        wcp = es_wc.enter_context(tc.tile_pool(name="wc", bufs=1,
                                               side="left"))
        wc = wcp.tile([128, c.DC, c.D], BF16, name="wc")
        nc.scalar.dma_start(
            out=wc[:], in_=w_cproj[:, :].rearrange("(i p) f -> p i f", p=128))

        es_at = ExitStack()
        atp = es_at.enter_context(tc.tile_pool(name="atp", bufs=1,
                                               side="left"))
        at = atp.tile([128, c.DC, c.T], BF16, name="at")

        with (
            tc.tile_pool(name="bconst", bufs=1) as bconst,
            tc.tile_pool(name="mask", bufs=1) as maskp,
            tc.tile_pool(name="pt", bufs=3) as ptp,
            tc.tile_pool(name="rec", bufs=3) as recp,
            tc.tile_pool(name="ps_qk", bufs=2, space="PSUM") as psqk,
            tc.tile_pool(name="ps_o", bufs=2, space="PSUM") as pso,
            tc.tile_pool(name="ps_bc", bufs=1, space="PSUM") as psbc,
        ):
            qidx = bconst.tile([128, c.T], F32)
            nc.sync.dma_start(out=qidx[:], in_=bcast(qidx_in))
            kofs = bconst.tile([128, c.KC], F32)
            nc.sync.dma_start(out=kofs[:], in_=kofs_in[:, :])

            # groups of SPG query slots, 512 queries wide; key-chunk pairs
            # (own-parity loc, peer-parity loc+KCH) share one QK psum, one
            # exp, and (in fp8) one DoubleRow AV matmul.  A chunk with
            # minimal covered slot s_min = loc//CPB only spans query columns
            # [max(s0,s_min)*BS ...), so late chunks run narrow.
            groups = [list(range(c.SPG * gi, c.SPG * (gi + 1)))
                      for gi in range(c.SLOTS // c.SPG)]

            # boundary masks per (group, loc-chunk) whose inclusion is
            # partial or parity-dependent
            masks = {}
            for gi, g in enumerate(groups):
                s0, s3 = g[0], g[-1]
                qsl = slice(s0 * c.BS, s0 * c.BS + 512)
                for kc in range(c.KC):
                    loc = kc % c.KCH
                    s_min = loc // c.CPB
                    if s0 <= s_min <= s3:
                        mk = maskp.tile([128, 512], BF16, tag=f"mk{gi}_{kc}",
                                        name=f"mk{gi}_{kc}")
                        nc.vector.tensor_scalar(
                            out=mk[:], in0=qidx[:, qsl],
                            scalar1=kofs[:, kc:kc + 1], scalar2=None,
                            op0=ALU.is_ge)
                        masks[(gi, kc)] = mk

            for jj in range(c.DC):
                for hp in range(c.HPB):
                    h = c.HPB * jj + hp
                    base = hp * 64
                    for gi, g in enumerate(groups):
                        s0, s3 = g[0], g[-1]
                        n_loc = (s3 + 1) * c.CPB
                        po = pso.tile([65, 512], F32, tag="po")
                        for loc in range(n_loc):
                            lo = max(s0, loc // c.CPB)
                            w = (s3 - lo + 1) * c.BS
                            ocol = (lo - s0) * c.BS
                            qsl = slice(lo * c.BS, (s3 + 1) * c.BS)
                            ps2 = psqk.tile([128, 2, 512], F32, tag="qk")
                            pt = ptp.tile([128, 2, 512], PT, tag="pt")
                            for ix in range(2):
                                kc = loc + ix * c.KCH
                                nc.tensor.matmul(
                                    ps2[:, ix, 0:w],
                                    ktp[base:base + 64, jj,
                                        kc * 128:(kc + 1) * 128],
                                    qtp[base:base + 64, jj, qsl],
                                    start=True, stop=True)
                            nc.scalar.activation(pt[:, :, 0:w],
                                                 ps2[:, :, 0:w], AF.Exp)
                            for ix in range(2):
                                kc = loc + ix * c.KCH
                                if (gi, kc) in masks:
                                    nc.vector.tensor_mul(
                                        pt[:, ix, 0:w], pt[:, ix, 0:w],
                                        masks[(gi, kc)][:, ocol:512])
                            if c.av_fp8:
                                nc.tensor.matmul(
                                    po[:, ocol:512], vtt[:, :, loc, h, :],
                                    pt[:, :, 0:w],
                                    start=(loc == 0), stop=(loc == n_loc - 1),
                                    perf_mode=mybir.MatmulPerfMode.DoubleRow)
                            else:
                                for ix in range(2):
                                    nc.tensor.matmul(
                                        po[:, ocol:512],
                                        vtt[:, ix, loc, h, :],
                                        pt[:, ix, 0:w],
                                        start=(loc == 0 and ix == 0),
                                        stop=(loc == n_loc - 1 and ix == 1))
                        # normalize by softmax denominator (row 64)
                        gq = slice(s0 * c.BS, s0 * c.BS + 512)
                        rec = recp.tile([1, 512], F32R, tag="rec")
                        with nc.allow_low_precision(
                                reason="softmax denom in f32r"):
                            nc.vector.reciprocal(rec[:], po[64:65, :])
                        bc = psbc.tile([64, 512], F32, tag="bc")
                        nc.tensor.matmul(bc[:], ones64[:], rec[:],
                                         start=True, stop=True)
                        bcs = recp.tile([64, 512], F32, tag="bcs")
                        nc.vector.tensor_copy(bcs[:], bc[:])
                        nc.vector.tensor_mul(
                            at[base:base + 64, jj, gq], po[0:64, :], bcs[:])

        es_kvq.close()

        # ================= phase C: c_proj + residual =================
        es_x2 = ExitStack()
        x2p = es_x2.enter_context(tc.tile_pool(name="x2p", bufs=1,
                                               side="right"))
        x2t = []
        with tc.tile_pool(name="ps_c", bufs=4, space="PSUM") as psc:
            for tb in range(c.TB):
                x2 = x2p.tile([128, c.D], F32, tag=f"x2_{tb}",
                              name=f"x2_{tb}")
                pss = [psc.tile([128, 512], F32, tag="ps", name=f"psc{q}") for q in range(2)]
                for i in range(c.DC):
                    for fh in range(2):
                        nc.tensor.matmul(
                            pss[fh][:], at[:, i, tb * 128:(tb + 1) * 128],
                            wc[:, i, fh * 512:(fh + 1) * 512],
                            start=(i == 0), stop=(i == c.DC - 1))
                for fh in range(2):
                    fsl = slice(fh * 512, (fh + 1) * 512)
                    nc.vector.tensor_add(x2[:, fsl], pss[fh][:],
                                         xloc[tb][:, fsl])
                x2t.append(x2)

        es_at.close()
        es_wc.close()
        es_per.close()

        # ================= phase D: LN2 + MLP =================
        with (
            tc.tile_pool(name="dconst", bufs=1) as dconst,
            tc.tile_pool(name="gt", bufs=1) as gtp,
            tc.tile_pool(name="wm", bufs=1) as wmp,
        ):
            bmp_b = dconst.tile([128, c.D], F32)
            nc.sync.dma_start(out=bmp_b[:], in_=bcast(bmp_in))
            bfc = dconst.tile([128, c.GB], F32)
            nc.sync.dma_start(out=bfc[:], in_=bfc_in[:, :])

            PMf = mybir.MatmulPerfMode.DoubleRow if c.fc_fp8 else None
            PMm = mybir.MatmulPerfMode.DoubleRow if c.mproj_fp8 else None
            NI2 = c.DC // 2 if c.fc_fp8 else c.DC    # fc contraction steps
            NG = c.GB // 2 if c.mproj_fp8 else c.GB  # mproj contraction steps
            inv_f = 1.0 / c.fscale
            inv_m = 1.0 / c.mscale
            gt = gtp.tile([128, c.GB, c.T], MPT, name="gt")
            x2b = [gtp.tile([128, c.D], F32, tag=f"x2b{tb}",
                            name=f"x2b{tb}") for tb in range(c.TB)]
            # mproj weights prefetched during LN2/fc on the sync queue so
            # they don't serialize behind the fc slab loads (scalar queue)
            wm_all = []
            for fh in range(2):
                wm = wmp.tile([128, c.GB, 512], MPT, tag=f"wm{fh}",
                              name=f"wm{fh}")
                nc.sync.dma_start(
                    out=wm[:],
                    in_=w_mproj[:, fh * 512:(fh + 1) * 512].rearrange(
                        "(g p) f -> p g f", p=128))
                wm_all.append(wm)
            with (
                tc.tile_pool(name="mtp", bufs=1) as mtp,
                tc.tile_pool(name="lnp2", bufs=2) as lnp2,
                tc.tile_pool(name="wf", bufs=2) as wfp,
                tc.tile_pool(name="ps_tr2", bufs=3, space="PSUM") as ps_tr2,
                tc.tile_pool(name="ps_g", bufs=2, space="PSUM") as psg,
            ):
                mt = mtp.tile([128, c.DC, c.T], FT, name="mt")
                layernorm_to(lambda tb: x2t[tb], c.TB, mt, lnp2, ps_tr2, "d")
                for tb in range(c.TB):
                    nc.vector.tensor_add(x2b[tb][:], x2t[tb][:], bmp_b[:])

                def msl(i, tsl):
                    if c.fc_fp8:
                        return mt[:, 2 * i:2 * i + 2, tsl]
                    return mt[:, i, tsl]

                # ---------------- fc + gelu ----------------
                wf = None
                for gb in range(c.GB):
                    if gb % 4 == 0:
                        wf = wfp.tile([128, c.DC, 512], FT, tag="wf",
                                      name=f"wf{gb}")
                        j = gb // 4
                        nc.scalar.dma_start(
                            out=wf[:],
                            in_=w_fc[:, j * 512:(j + 1) * 512].rearrange(
                                "(i p) f -> p i f", p=128))
                    gl = (gb % 4) * 128
                    ps = psg.tile([128, 1024], F32, tag="ps")
                    for i in range(NI2):
                        wfs = (wf[:, 2 * i:2 * i + 2, gl:gl + 128]
                               if c.fc_fp8 else wf[:, i, gl:gl + 128])
                        for th in range(2):
                            nc.tensor.matmul(
                                ps[:, th * 512:(th + 1) * 512], wfs,
                                msl(i, slice(th * 512, (th + 1) * 512)),
                                start=(i == 0), stop=(i == NI2 - 1),
                                perf_mode=PMf)
                    nc.scalar.activation(
                        gt[:, gb, :], ps[:], AF.Gelu_apprx_tanh,
                        bias=bfc[:, gb:gb + 1], scale=inv_f)

            # ---------------- mproj + residual ----------------
            with (
                tc.tile_pool(name="yout", bufs=3) as yop,
                tc.tile_pool(name="mo", bufs=3) as mop,
                tc.tile_pool(name="ps_m", bufs=4, space="PSUM") as psm,
            ):
                for tb in range(c.TB):
                    tbs = slice(tb * 128, (tb + 1) * 128)
                    yo = yop.tile([128, c.D], F32, tag="yo")
                    pss = [psm.tile([128, 512], F32, tag="ps", name=f"psm{q}")
                           for q in range(2)]
                    for g in range(NG):
                        gts = (gt[:, 2 * g:2 * g + 2, tbs]
                               if c.mproj_fp8 else gt[:, g, tbs])
                        for fh in range(2):
                            wms = (wm_all[fh][:, 2 * g:2 * g + 2, :]
                                   if c.mproj_fp8 else wm_all[fh][:, g, :])
                            nc.tensor.matmul(
                                pss[fh][:], gts, wms,
                                start=(g == 0), stop=(g == NG - 1),
                                perf_mode=PMm)
                    for fh in range(2):
                        fsl = slice(fh * 512, (fh + 1) * 512)
                        if c.mproj_fp8:
                            mo = mop.tile([128, 512], F32, tag="mo")
                            nc.scalar.activation(mo[:], pss[fh][:],
                                                 AF.Identity, scale=inv_m)
                            nc.vector.tensor_add(yo[:, fsl], mo[:],
                                                 x2b[tb][:, fsl])
                        else:
                            nc.vector.tensor_add(yo[:, fsl], pss[fh][:],
                                                 x2b[tb][:, fsl])
                    nc.sync.dma_start(
                        out=y_out[tb * 128:(tb + 1) * 128, :], in_=yo[:])

        es_x2.close()

    nc.compile()
    return nc


def core_rows(cfg, half):
    """absolute sequence rows owned by a core with parity half"""
    c = cfg
    loc = np.arange(c.T)
    return (2 * (loc // c.BS) + half) * c.BS + loc % c.BS


def make_core_inputs(cfg: Cfg, x, ln1_w, ln1_b, W_attn, b_attn, W_cproj,
                     b_cproj, ln2_w, ln2_b, W_fc, b_fc, W_mproj, b_mproj):
    """Split full inputs into one in_map per core."""
    c = cfg
    f32 = np.float32
    qt = ml_dtypes.float8_e4m3fn if c.qkv_fp8 else ml_dtypes.bfloat16

    # fold LN1 affine + query scale into W_attn / b_attn
    ln1_w = np.asarray(ln1_w, f32)
    ln1_b = np.asarray(ln1_b, f32)
    Wa = np.asarray(W_attn, f32) * ln1_w[:, None]
    ba = np.asarray(b_attn, f32) + ln1_b @ np.asarray(W_attn, f32)
    qs = 1.0 / math.sqrt(c.HD)
    Wa = Wa.copy()
    Wa[:, :c.D] *= qs
    ba = ba.copy()
    ba[:c.D] *= qs
    Wa_dev = (Wa * c.wscale).astype(qt)

    # fold LN2 affine into W_fc / b_fc
    ln2_w = np.asarray(ln2_w, f32)
    ln2_b = np.asarray(ln2_b, f32)
    Wf = np.asarray(W_fc, f32) * ln2_w[:, None]
    bf = np.asarray(b_fc, f32) + ln2_b @ np.asarray(W_fc, f32)

    fc_dt = ml_dtypes.float8_e4m3fn if c.fc_fp8 else ml_dtypes.bfloat16
    mp_dt = ml_dtypes.float8_e4m3fn if c.mproj_fp8 else ml_dtypes.bfloat16
    shared = {
        "w_attn": np.ascontiguousarray(Wa_dev),
        "w_cproj": np.ascontiguousarray(W_cproj).astype(ml_dtypes.bfloat16),
        "w_fc": np.ascontiguousarray(Wf * c.fscale).astype(fc_dt),
        "w_mproj": np.ascontiguousarray(
            np.asarray(W_mproj, f32) * c.mscale).astype(mp_dt),
        "bv": np.ascontiguousarray(ba[2 * c.D:3 * c.D]).reshape(1, c.D),
        "bcp": np.ascontiguousarray(b_cproj, f32).reshape(1, c.D),
        "bmp": np.ascontiguousarray(b_mproj, f32).reshape(1, c.D),
        "bfc": np.ascontiguousarray(bf.reshape(c.GB, 128).T),
        "battn_qk": np.ascontiguousarray(
            ba[:2 * c.D].reshape(2 * c.DC, 128).T),
    }

    x = np.asarray(x, f32)
    in_maps = []
    for core in range(c.n_cores):
        b, half = core // 2, core % 2
        own = core_rows(c, half)
        peer = core_rows(c, 1 - half)
        perm = np.concatenate([own, peer])
        m = dict(shared)
        m["x"] = np.ascontiguousarray(x[b][perm])
        m["qidx"] = own.astype(f32).reshape(1, c.T)
        kofs = np.empty((128, c.KC), f32)
        for kc in range(c.KC):
            kofs[:, kc] = perm[kc * 128 + np.arange(128)]
        m["kofs"] = kofs
        in_maps.append(m)
    return in_maps


_NC_CACHE = {}


def get_nc(cfg: Cfg):
    key = (cfg.B, cfg.S, cfg.D, cfg.H, cfg.F, cfg.qkv_fp8, cfg.fc_fp8,
           cfg.mproj_fp8, cfg.av_fp8, cfg.BS)
    if key not in _NC_CACHE:
        _NC_CACHE[key] = build(cfg)
    return _NC_CACHE[key]


def kernel(**inputs) -> np.ndarray:
    from concourse.bass_utils import run_bass_kernel_spmd

    cfg = Cfg()
    nc = get_nc(cfg)
    in_maps = make_core_inputs(cfg, **inputs)
    res = run_bass_kernel_spmd(nc, in_maps, core_ids=list(range(cfg.n_cores)))
    B, S, D = cfg.B, cfg.S, cfg.D
    out = np.empty((B, S, D), np.float32)
    for core in range(cfg.n_cores):
        b, half = core // 2, core % 2
        out[b, core_rows(cfg, half), :] = res.results[core]["y"]
    return out


# revision 28
# speedup vs baseline: 2.0478x; 1.0993x over previous
"""Single transformer block on 8 NeuronCores — collective-free.

Sharding: core c = (batch b=c//2, parity p=c%2). Each core receives the FULL
sequence of its batch, permuted to [own-stripe | peer-stripe] order, and
recomputes K and V for all 2048 tokens locally — cheaper than the pairwise
AllGather it replaces (~55us extra PE vs ~270us of collective time) and it
deletes all DRAM bounce traffic.  Q / attention / c_proj / MLP cover only the
core's 1024 own (striped) tokens.

Tricks:
  - LayerNorm affine (w, b) folds host-side into the following matmul
    weights/bias, so on-chip LN is just (x - mean) * rsqrt(var + eps).
  - The 1/sqrt(hd) query scale folds host-side into W_q / b_q.
  - V is built directly in [128 key, KC, H, 65] layout with a ones column at
    65, so AV yields the softmax denominator for free and per-head V slices
    are zero-copy views.
  - Scores are computed transposed S^T[k, q]; causal mask is a 0/1 multiply
    on P = exp(S) (finite, exact).
  - AV accumulates a whole 512-query group into one [65, 512] PSUM bank:
    chunks common to both 256-slots run 512-wide, diagonal-extra chunks run
    256-wide into the upper half.
  - Optional fp8 (e4m3) QKV projection with DoubleRow matmuls (2x PE rate,
    half the instructions); weights are pre-scaled x64 host-side so 0.02-std
    values stay out of the fp8 subnormal range, undone at eviction.
"""

import math
from contextlib import ExitStack

import numpy as np
import ml_dtypes

import concourse.bacc as bacc
import concourse.bass as bass
import concourse.mybir as mybir
import concourse.tile as tile
from concourse.masks import make_identity

F32 = mybir.dt.float32
F32R = mybir.dt.float32r
BF16 = mybir.dt.bfloat16
F8 = mybir.dt.float8e4
AF = mybir.ActivationFunctionType
ALU = mybir.AluOpType

EPS = 1e-5


class Cfg:
    def __init__(self, B=4, S=2048, D=1024, H=16, F=4096, n_cores=8,
                 qkv_fp8=True, fc_fp8=False, mproj_fp8=True, av_fp8=True,
                 BS=128):
        self.B, self.S, self.D, self.H, self.F = B, S, D, H, F
        self.n_cores = n_cores
        assert n_cores == 2 * B
        self.HD = D // H
        assert self.HD == 64
        self.T = S // 2            # tokens owned per core
        self.KT = S // 128         # token 128-tiles, full sequence
        self.TB = self.T // 128    # token 128-tiles, local
        self.DC = D // 128         # contraction chunks over D
        self.QF = 512              # free-dim tile for projection matmuls
        self.KC = S // 128         # key 128-chunks over full sequence
        self.GB = F // 128         # MLP hidden 128-blocks
        self.HPB = 128 // self.HD  # heads per 128-feature block (=2)
        self.BS = BS               # stripe block (q-slot) size
        self.SLOTS = self.T // self.BS
        self.SPG = 512 // self.BS  # slots per 512-wide attention group
        self.KCH = self.KC // 2    # chunks per parity half
        self.CPB = self.BS // 128  # key chunks per stripe block
        self.qkv_fp8 = qkv_fp8
        self.fc_fp8 = fc_fp8
        self.mproj_fp8 = mproj_fp8
        self.av_fp8 = av_fp8
        self.wscale = 64.0 if qkv_fp8 else 1.0
        self.fscale = 64.0 if fc_fp8 else 1.0
        self.mscale = 64.0 if mproj_fp8 else 1.0


def chunk_absblk(c, kc):
    # conservative absolute stripe-block index covered by key chunk kc
    parity = kc // c.KCH
    loc = kc % c.KCH
    return 2 * (loc // (c.BS // 128)) + parity


def build(cfg: Cfg):
    c = cfg
    QT = F8 if c.qkv_fp8 else BF16
    FT = F8 if c.fc_fp8 else BF16
    MPT = F8 if c.mproj_fp8 else BF16
    VT = F8 if c.av_fp8 else BF16
    PT = VT
    nc = bacc.Bacc(None, target_bir_lowering=False)

    # ---------------- I/O ----------------
    x_in = nc.dram_tensor("x", [c.S, c.D], F32, kind="ExternalInput")
    w_attn = nc.dram_tensor("w_attn", [c.D, 3 * c.D], QT, kind="ExternalInput")
    w_cproj = nc.dram_tensor("w_cproj", [c.D, c.D], BF16, kind="ExternalInput")
    w_fc = nc.dram_tensor("w_fc", [c.D, c.F], FT, kind="ExternalInput")
    w_mproj = nc.dram_tensor("w_mproj", [c.F, c.D], MPT,
                             kind="ExternalInput")
    battn_qk_in = nc.dram_tensor("battn_qk", [128, 2 * c.DC], F32,
                                 kind="ExternalInput")
    bv_in = nc.dram_tensor("bv", [1, c.D], F32, kind="ExternalInput")
    bcp_in = nc.dram_tensor("bcp", [1, c.D], F32, kind="ExternalInput")
    bmp_in = nc.dram_tensor("bmp", [1, c.D], F32, kind="ExternalInput")
    bfc_in = nc.dram_tensor("bfc", [128, c.GB], F32, kind="ExternalInput")
    qidx_in = nc.dram_tensor("qidx", [1, c.T], F32, kind="ExternalInput")
    kofs_in = nc.dram_tensor("kofs", [128, c.KC], F32, kind="ExternalInput")
    y_out = nc.dram_tensor("y", [c.T, c.D], F32, kind="ExternalOutput")

    def bcast(dram, p=128):
        # partition-broadcast DMA source: read row 0 for every partition
        return bass.AP(tensor=dram, offset=0, ap=[[0, p], [1, dram.shape[1]]])

    with tile.TileContext(nc) as tc, ExitStack() as es:
        gconst = es.enter_context(tc.tile_pool(name="gconst", bufs=1))
        ident = gconst.tile([128, 128], F32)
        make_identity(nc, ident[:])
        eps_t = gconst.tile([128, 1], F32)
        nc.vector.memset(eps_t[:], EPS)
        ones64_f = gconst.tile([1, 64], F32)
        nc.vector.memset(ones64_f[:], 1.0)
        ones64 = gconst.tile([1, 64], F32R)
        nc.vector.tensor_copy(ones64[:], ones64_f[:])

        def layernorm_to(get_src, n_tiles, dest, lnp, ps_tr, tag,
                         interleave=None):
            """normalize token tiles and write feature-major into dest
            [128, DC, n_tiles*128].  get_src(tb) -> token-major [128, D] tile.
            interleave(g) is called after every 4th tile to emit consumer
            work early (keeps PE fed in emission order)."""
            for tb in range(n_tiles):
                src = get_src(tb)
                st = lnp.tile([128, 2, 6], F32, tag=f"{tag}st")
                for sg in range(2):
                    nc.vector.bn_stats(
                        out=st[:, sg, :], in_=src[:, sg * 512:(sg + 1) * 512])
                mv = lnp.tile([128, 2], F32, tag=f"{tag}mv")
                nc.vector.bn_aggr(out=mv[:], in_=st[:])
                sd = lnp.tile([128, 1], F32, tag=f"{tag}sd")
                nc.scalar.activation(sd[:], mv[:, 1:2], AF.Sqrt,
                                     bias=eps_t[:, 0:1])
                rs = lnp.tile([128, 1], F32, tag=f"{tag}rs")
                nc.vector.reciprocal(rs[:], sd[:])
                nrm = lnp.tile([128, c.D], F32, tag=f"{tag}n")
                nc.vector.tensor_scalar(
                    out=nrm[:], in0=src[:], scalar1=mv[:, 0:1],
                    scalar2=rs[:, 0:1], op0=ALU.subtract, op1=ALU.mult)
                for i2 in range(c.DC // 4):
                    pt = ps_tr.tile([128, 512], F32, tag=f"{tag}tr")
                    for j in range(4):
                        ch = 4 * i2 + j
                        nc.tensor.matmul(
                            pt[:, j * 128:(j + 1) * 128],
                            nrm[:, ch * 128:(ch + 1) * 128], ident[:],
                            is_transpose=True, start=(j == 0), stop=(j == 3))
                    nc.scalar.activation(
                        dest[:, 4 * i2:4 * i2 + 4, tb * 128:(tb + 1) * 128],
                        pt[:], AF.Identity)
                if interleave is not None and tb % 4 == 3:
                    interleave(tb // 4)

        # ---------------- persistent activations ----------------
        es_per = ExitStack()
        xloc = []
        xlp = es_per.enter_context(tc.tile_pool(name="xloc", bufs=1,
                                                side="left"))
        for tb in range(c.TB):
            t = xlp.tile([128, c.D], F32, tag=f"x{tb}", name=f"x{tb}")
            nc.sync.dma_start(out=t[:], in_=x_in[tb * 128:(tb + 1) * 128, :])
            xloc.append(t)

        es_kvq = ExitStack()
        kvqp = es_kvq.enter_context(tc.tile_pool(name="kvq", bufs=1,
                                                 side="right"))
        ktp = kvqp.tile([128, c.DC, c.S], BF16, name="ktp")
        vtt = kvqp.tile([128, 2, c.KCH, c.H, 65], VT, name="vtt")
        qtp = kvqp.tile([128, c.DC, c.T], BF16, name="qtp")
        nc.vector.memset(vtt[:, :, :, :, 64:65], 1.0)

        # ================= phase A: LN1 + QKV =================
        es_ht = ExitStack()
        htp = es_ht.enter_context(tc.tile_pool(name="htp", bufs=1))
        ht = htp.tile([128, c.DC, c.S], QT, name="ht")

        with (
            tc.tile_pool(name="aconst", bufs=1) as aconst,
            tc.tile_pool(name="xs", bufs=2) as xsp,
            tc.tile_pool(name="lnp", bufs=2) as lnp,
            tc.tile_pool(name="wa", bufs=2) as wap,
            tc.tile_pool(name="ko", bufs=4) as kop,
            tc.tile_pool(name="ps_tr", bufs=2, space="PSUM") as ps_tr,
            tc.tile_pool(name="ps_mm", bufs=2, space="PSUM") as psmm,
        ):
            battn_qk = aconst.tile([128, 2 * c.DC], F32)
            nc.sync.dma_start(out=battn_qk[:], in_=battn_qk_in[:, :])
            bv_b = aconst.tile([128, c.D], F32)
            nc.sync.dma_start(out=bv_b[:], in_=bcast(bv_in))
            bcp_b = aconst.tile([128, c.D], F32)
            nc.sync.dma_start(out=bcp_b[:], in_=bcast(bcp_in))

            wk = wap.tile([128, c.DC, c.D], QT, tag="wa", name="wk")
            nc.scalar.dma_start(
                out=wk[:],
                in_=w_attn[:, c.D:2 * c.D].rearrange("(i p) f -> p i f",
                                                     p=128))
            wv = wap.tile([128, c.DC, c.D], QT, tag="wa", name="wv")
            nc.scalar.dma_start(
                out=wv[:],
                in_=w_attn[:, 2 * c.D:3 * c.D].rearrange("(i p) f -> p i f",
                                                         p=128))

            inv_w = 1.0 / c.wscale
            NI = c.DC // 2 if c.qkv_fp8 else c.DC  # contraction steps

            def wsl(wslab, i, fsl):
                # weight slab contraction-step slice (pair of chunks in fp8)
                if c.qkv_fp8:
                    return wslab[:, 2 * i:2 * i + 2, fsl]
                return wslab[:, i, fsl]

            def hsl(i, tsl):
                if c.qkv_fp8:
                    return ht[:, 2 * i:2 * i + 2, tsl]
                return ht[:, i, tsl]

            PM = mybir.MatmulPerfMode.DoubleRow if c.qkv_fp8 else None

            def v_chunk(g):
                # V for these 4 token tiles; lhsT (ht slice) shared across
                # both feature halves so ldweights is loaded once per step
                for tb in range(4 * g, 4 * g + 4):
                    tbs = slice(tb * 128, (tb + 1) * 128)
                    pss = [psmm.tile([128, 512], F32, tag="ps", name=f"psv{q}")
                           for q in range(2)]
                    for i in range(NI):
                        for vh in range(2):
                            nc.tensor.matmul(
                                pss[vh][:], hsl(i, tbs),
                                wsl(wv, i, slice(vh * 512, (vh + 1) * 512)),
                                start=(i == 0), stop=(i == NI - 1),
                                perf_mode=PM)
                    for vh in range(2):
                        fsl = slice(vh * 512, (vh + 1) * 512)
                        dst = vtt[:, tb // c.KCH, tb % c.KCH,
                                  vh * 8:(vh + 1) * 8, 0:64]
                        if c.qkv_fp8:
                            vo = kop.tile([128, 512], BF16, tag="vo")
                            nc.scalar.activation(vo[:], pss[vh][:],
                                                 AF.Identity, scale=inv_w)
                            nc.vector.tensor_add(dst, vo[:], bv_b[:, fsl])
                        else:
                            nc.vector.tensor_add(dst, pss[vh][:],
                                                 bv_b[:, fsl])

            def get_src(tb):
                if tb < c.TB:
                    return xloc[tb]
                t = xsp.tile([128, c.D], F32, tag="xs")
                nc.sync.dma_start(out=t[:],
                                  in_=x_in[tb * 128:(tb + 1) * 128, :])
                return t

            layernorm_to(get_src, c.KT, ht, lnp, ps_tr, "a",
                         interleave=v_chunk)

            # ---- k^T pass: lhsT (w chunk) shared across 4 token slices ----
            for m in range(c.DC):
                msl = slice(m * 128, (m + 1) * 128)
                pss = [psmm.tile([128, 2, 512], F32, tag="pk",
                                 name=f"psk{q}") for q in range(2)]
                for i in range(NI):
                    for th in range(4):
                        nc.tensor.matmul(
                            pss[th // 2][:, th % 2, :], wsl(wk, i, msl),
                            hsl(i, slice(th * 512, (th + 1) * 512)),
                            start=(i == 0), stop=(i == NI - 1),
                            perf_mode=PM)
                for q in range(2):
                    nc.scalar.activation(
                        ktp[:, m, q * 1024:(q + 1) * 1024], pss[q][:],
                        AF.Identity,
                        bias=battn_qk[:, c.DC + m:c.DC + m + 1], scale=inv_w)

            # ---- q^T pass (local tokens only; scale folded host-side) ----
            wq = wap.tile([128, c.DC, c.D], QT, tag="wa", name="wq")
            nc.scalar.dma_start(
                out=wq[:],
                in_=w_attn[:, 0:c.D].rearrange("(i p) f -> p i f", p=128))
            for m in range(c.DC):
                msl = slice(m * 128, (m + 1) * 128)
                psq = psmm.tile([128, 2, 512], F32, tag="pk", name="psq")
                for i in range(NI):
                    for th in range(2):
                        nc.tensor.matmul(
                            psq[:, th, :], wsl(wq, i, msl),
                            hsl(i, slice(th * 512, (th + 1) * 512)),
                            start=(i == 0), stop=(i == NI - 1),
                            perf_mode=PM)
                nc.scalar.activation(
                    qtp[:, m, :], psq[:],
                    AF.Identity, bias=battn_qk[:, m:m + 1], scale=inv_w)

            # fold the c_proj bias into the residual copy of x, in place
            for tb in range(c.TB):
                nc.vector.tensor_add(xloc[tb][:], xloc[tb][:], bcp_b[:])

        es_ht.close()

        # ================= phase B: attention =================
        # prefetch c_proj weights during attention
        es_wc = ExitStack()
        wcp = es_wc.enter_context(tc.tile_pool(name="wc", bufs=1,
                                               side="left"))
        wc = wcp.tile([128, c.DC, c.D], BF16, name="wc")
        nc.scalar.dma_start(
            out=wc[:], in_=w_cproj[:, :].rearrange("(i p) f -> p i f", p=128))

        es_at = ExitStack()
        atp = es_at.enter_context(tc.tile_pool(name="atp", bufs=1,
                                               side="left"))
        at = atp.tile([128, c.DC, c.T], BF16, name="at")

        with (
            tc.tile_pool(name="bconst", bufs=1) as bconst,
            tc.tile_pool(name="mask", bufs=1) as maskp,
            tc.tile_pool(name="pt", bufs=3) as ptp,
            tc.tile_pool(name="rec", bufs=3) as recp,
            tc.tile_pool(name="ps_qk", bufs=2, space="PSUM") as psqk,
            tc.tile_pool(name="ps_o", bufs=2, space="PSUM") as pso,
            tc.tile_pool(name="ps_bc", bufs=1, space="PSUM") as psbc,
        ):
            qidx = bconst.tile([128, c.T], F32)
            nc.sync.dma_start(out=qidx[:], in_=bcast(qidx_in))
            kofs = bconst.tile([128, c.KC], F32)
            nc.sync.dma_start(out=kofs[:], in_=kofs_in[:, :])

            # groups of SPG query slots, 512 queries wide; key-chunk pairs
            # (own-parity loc, peer-parity loc+KCH) share one QK psum, one
            # exp, and (in fp8) one DoubleRow AV matmul.  A chunk with
            # minimal covered slot s_min = loc//CPB only spans query columns
            # [max(s0,s_min)*BS ...), so late chunks run narrow.
            groups = [list(range(c.SPG * gi, c.SPG * (gi + 1)))
                      for gi in range(c.SLOTS // c.SPG)]

            # A chunk's mask differs from all-ones only in the diagonal
            # slot s_min (queries in later slots are >= every key of the
            # chunk for either parity), so one [128, BS] mask per chunk.
            masks = {}
            for kc in range(c.KC):
                s_min = (kc % c.KCH) // c.CPB
                qsl = slice(s_min * c.BS, (s_min + 1) * c.BS)
                mk = maskp.tile([128, c.BS], BF16, tag=f"mk{kc}",
                                name=f"mk{kc}")
                nc.vector.tensor_scalar(
                    out=mk[:], in0=qidx[:, qsl],
                    scalar1=kofs[:, kc:kc + 1], scalar2=None,
                    op0=ALU.is_ge)
                masks[kc] = mk

            for jj in range(c.DC):
                for hp in range(c.HPB):
                    h = c.HPB * jj + hp
                    base = hp * 64
                    for gi, g in enumerate(groups):
                        s0, s3 = g[0], g[-1]
                        n_loc = (s3 + 1) * c.CPB
                        po = pso.tile([65, 512], F32, tag="po")
                        for loc in range(n_loc):
                            lo = max(s0, loc // c.CPB)
                            w = (s3 - lo + 1) * c.BS
                            ocol = (lo - s0) * c.BS
                            qsl = slice(lo * c.BS, (s3 + 1) * c.BS)
                            ps2 = psqk.tile([128, 2, 512], F32, tag="qk")
                            pt = ptp.tile([128, 2, 512], PT, tag="pt")
                            for ix in range(2):
                                kc = loc + ix * c.KCH
                                nc.tensor.matmul(
                                    ps2[:, ix, 0:w],
                                    ktp[base:base + 64, jj,
                                        kc * 128:(kc + 1) * 128],
                                    qtp[base:base + 64, jj, qsl],
                                    start=True, stop=True)
                            nc.scalar.activation(pt[:, :, 0:w],
                                                 ps2[:, :, 0:w], AF.Exp)
                            if loc // c.CPB >= s0:
                                for ix in range(2):
                                    kc = loc + ix * c.KCH
                                    nc.vector.tensor_mul(
                                        pt[:, ix, 0:c.BS], pt[:, ix, 0:c.BS],
                                        masks[kc][:])
                            if c.av_fp8:
                                nc.tensor.matmul(
                                    po[:, ocol:512], vtt[:, :, loc, h, :],
                                    pt[:, :, 0:w],
                                    start=(loc == 0), stop=(loc == n_loc - 1),
                                    perf_mode=mybir.MatmulPerfMode.DoubleRow)
                            else:
                                for ix in range(2):
                                    nc.tensor.matmul(
                                        po[:, ocol:512],
                                        vtt[:, ix, loc, h, :],
                                        pt[:, ix, 0:w],
                                        start=(loc == 0 and ix == 0),
                                        stop=(loc == n_loc - 1 and ix == 1))
                        # normalize by softmax denominator (row 64)
                        gq = slice(s0 * c.BS, s0 * c.BS + 512)
                        rec = recp.tile([1, 512], F32R, tag="rec")
                        with nc.allow_low_precision(
                                reason="softmax denom in f32r"):
                            nc.vector.reciprocal(rec[:], po[64:65, :])
                        bc = psbc.tile([64, 512], F32, tag="bc")
                        nc.tensor.matmul(bc[:], ones64[:], rec[:],
                                         start=True, stop=True)
                        bcs = recp.tile([64, 512], F32, tag="bcs")
                        nc.vector.tensor_copy(bcs[:], bc[:])
                        nc.vector.tensor_mul(
                            at[base:base + 64, jj, gq], po[0:64, :], bcs[:])

        es_kvq.close()

        # ================= phase C: c_proj + residual =================
        es_x2 = ExitStack()
        x2p = es_x2.enter_context(tc.tile_pool(name="x2p", bufs=1,
                                               side="right"))
        x2t = []
        with tc.tile_pool(name="ps_c", bufs=4, space="PSUM") as psc:
            for tb in range(c.TB):
                x2 = x2p.tile([128, c.D], F32, tag=f"x2_{tb}",
                              name=f"x2_{tb}")
                pss = [psc.tile([128, 512], F32, tag="ps", name=f"psc{q}") for q in range(2)]
                for i in range(c.DC):
                    for fh in range(2):
                        nc.tensor.matmul(
                            pss[fh][:], at[:, i, tb * 128:(tb + 1) * 128],
                            wc[:, i, fh * 512:(fh + 1) * 512],
                            start=(i == 0), stop=(i == c.DC - 1))
                for fh in range(2):
                    fsl = slice(fh * 512, (fh + 1) * 512)
                    nc.vector.tensor_add(x2[:, fsl], pss[fh][:],
                                         xloc[tb][:, fsl])
                x2t.append(x2)

        es_at.close()
        es_wc.close()
        es_per.close()

        # ================= phase D: LN2 + MLP =================
        with (
            tc.tile_pool(name="dconst", bufs=1) as dconst,
            tc.tile_pool(name="gt", bufs=1) as gtp,
            tc.tile_pool(name="wm", bufs=1) as wmp,
        ):
            bmp_b = dconst.tile([128, c.D], F32)
            nc.sync.dma_start(out=bmp_b[:], in_=bcast(bmp_in))
            bfc = dconst.tile([128, c.GB], F32)
            nc.sync.dma_start(out=bfc[:], in_=bfc_in[:, :])

            PMf = mybir.MatmulPerfMode.DoubleRow if c.fc_fp8 else None
            PMm = mybir.MatmulPerfMode.DoubleRow if c.mproj_fp8 else None
            NI2 = c.DC // 2 if c.fc_fp8 else c.DC    # fc contraction steps
            NG = c.GB // 2 if c.mproj_fp8 else c.GB  # mproj contraction steps
            inv_f = 1.0 / c.fscale
            inv_m = 1.0 / c.mscale
            gt = gtp.tile([128, c.GB, c.T], MPT, name="gt")
            x2b = [gtp.tile([128, c.D], F32, tag=f"x2b{tb}",
                            name=f"x2b{tb}") for tb in range(c.TB)]
            # mproj weights prefetched during LN2/fc on the sync queue so
            # they don't serialize behind the fc slab loads (scalar queue)
            wm_all = []
            for fh in range(2):
                wm = wmp.tile([128, c.GB, 512], MPT, tag=f"wm{fh}",
                              name=f"wm{fh}")
                nc.sync.dma_start(
                    out=wm[:],
                    in_=w_mproj[:, fh * 512:(fh + 1) * 512].rearrange(
                        "(g p) f -> p g f", p=128))
                wm_all.append(wm)
            with (
                tc.tile_pool(name="mtp", bufs=1) as mtp,
                tc.tile_pool(name="lnp2", bufs=2) as lnp2,
                tc.tile_pool(name="wf", bufs=2) as wfp,
                tc.tile_pool(name="ps_tr2", bufs=3, space="PSUM") as ps_tr2,
                tc.tile_pool(name="ps_g", bufs=2, space="PSUM") as psg,
            ):
                mt = mtp.tile([128, c.DC, c.T], FT, name="mt")
                layernorm_to(lambda tb: x2t[tb], c.TB, mt, lnp2, ps_tr2, "d")
                for tb in range(c.TB):
                    nc.vector.tensor_add(x2b[tb][:], x2t[tb][:], bmp_b[:])

                def msl(i, tsl):
                    if c.fc_fp8:
                        return mt[:, 2 * i:2 * i + 2, tsl]
                    return mt[:, i, tsl]

                # ---------------- fc + gelu ----------------
                wf = None
                for gb in range(c.GB):
                    if gb % 4 == 0:
                        wf = wfp.tile([128, c.DC, 512], FT, tag="wf",
                                      name=f"wf{gb}")
                        j = gb // 4
                        nc.scalar.dma_start(
                            out=wf[:],
                            in_=w_fc[:, j * 512:(j + 1) * 512].rearrange(
                                "(i p) f -> p i f", p=128))
                    gl = (gb % 4) * 128
                    ps = psg.tile([128, 1024], F32, tag="ps")
                    for i in range(NI2):
                        wfs = (wf[:, 2 * i:2 * i + 2, gl:gl + 128]
                               if c.fc_fp8 else wf[:, i, gl:gl + 128])
                        for th in range(2):
                            nc.tensor.matmul(
                                ps[:, th * 512:(th + 1) * 512], wfs,
                                msl(i, slice(th * 512, (th + 1) * 512)),
                                start=(i == 0), stop=(i == NI2 - 1),
                                perf_mode=PMf)
                    nc.scalar.activation(
                        gt[:, gb, :], ps[:], AF.Gelu_apprx_tanh,
                        bias=bfc[:, gb:gb + 1], scale=inv_f)

            # ---------------- mproj + residual ----------------
            with (
                tc.tile_pool(name="yout", bufs=3) as yop,
                tc.tile_pool(name="mo", bufs=3) as mop,
                tc.tile_pool(name="ps_m", bufs=4, space="PSUM") as psm,
            ):
                for tb in range(c.TB):
                    tbs = slice(tb * 128, (tb + 1) * 128)
                    yo = yop.tile([128, c.D], F32, tag="yo")
                    pss = [psm.tile([128, 512], F32, tag="ps", name=f"psm{q}")
                           for q in range(2)]
                    for g in range(NG):
                        gts = (gt[:, 2 * g:2 * g + 2, tbs]
                               if c.mproj_fp8 else gt[:, g, tbs])
                        for fh in range(2):
                            wms = (wm_all[fh][:, 2 * g:2 * g + 2, :]
                                   if c.mproj_fp8 else wm_all[fh][:, g, :])
                            nc.tensor.matmul(
                                pss[fh][:], gts, wms,
                                start=(g == 0), stop=(g == NG - 1),
                                perf_mode=PMm)
                    for fh in range(2):
                        fsl = slice(fh * 512, (fh + 1) * 512)
                        if c.mproj_fp8:
                            mo = mop.tile([128, 512], F32, tag="mo")
                            nc.scalar.activation(mo[:], pss[fh][:],
                                                 AF.Identity, scale=inv_m)
                            nc.vector.tensor_add(yo[:, fsl], mo[:],
                                                 x2b[tb][:, fsl])
                        else:
                            nc.vector.tensor_add(yo[:, fsl], pss[fh][:],
                                                 x2b[tb][:, fsl])
                    nc.sync.dma_start(
                        out=y_out[tb * 128:(tb + 1) * 128, :], in_=yo[:])

        es_x2.close()

    nc.compile()
    return nc


def core_rows(cfg, half):
    """absolute sequence rows owned by a core with parity half"""
    c = cfg
    loc = np.arange(c.T)
    return (2 * (loc // c.BS) + half) * c.BS + loc % c.BS


def make_core_inputs(cfg: Cfg, x, ln1_w, ln1_b, W_attn, b_attn, W_cproj,
                     b_cproj, ln2_w, ln2_b, W_fc, b_fc, W_mproj, b_mproj):
    """Split full inputs into one in_map per core."""
    c = cfg
    f32 = np.float32
    qt = ml_dtypes.float8_e4m3fn if c.qkv_fp8 else ml_dtypes.bfloat16

    # fold LN1 affine + query scale into W_attn / b_attn
    ln1_w = np.asarray(ln1_w, f32)
    ln1_b = np.asarray(ln1_b, f32)
    Wa = np.asarray(W_attn, f32) * ln1_w[:, None]
    ba = np.asarray(b_attn, f32) + ln1_b @ np.asarray(W_attn, f32)
    qs = 1.0 / math.sqrt(c.HD)
    Wa = Wa.copy()
    Wa[:, :c.D] *= qs
    ba = ba.copy()
    ba[:c.D] *= qs
    Wa_dev = (Wa * c.wscale).astype(qt)

    # fold LN2 affine into W_fc / b_fc
    ln2_w = np.asarray(ln2_w, f32)
    ln2_b = np.asarray(ln2_b, f32)
    Wf = np.asarray(W_fc, f32) * ln2_w[:, None]
    bf = np.asarray(b_fc, f32) + ln2_b @ np.asarray(W_fc, f32)

    fc_dt = ml_dtypes.float8_e4m3fn if c.fc_fp8 else ml_dtypes.bfloat16
    mp_dt = ml_dtypes.float8_e4m3fn if c.mproj_fp8 else ml_dtypes.bfloat16
    shared = {
        "w_attn": np.ascontiguousarray(Wa_dev),
        "w_cproj": np.ascontiguousarray(W_cproj).astype(ml_dtypes.bfloat16),
        "w_fc": np.ascontiguousarray(Wf * c.fscale).astype(fc_dt),
        "w_mproj": np.ascontiguousarray(
            np.asarray(W_mproj, f32) * c.mscale).astype(mp_dt),
        "bv": np.ascontiguousarray(ba[2 * c.D:3 * c.D]).reshape(1, c.D),
        "bcp": np.ascontiguousarray(b_cproj, f32).reshape(1, c.D),
        "bmp": np.ascontiguousarray(b_mproj, f32).reshape(1, c.D),
        "bfc": np.ascontiguousarray(bf.reshape(c.GB, 128).T),
        "battn_qk": np.ascontiguousarray(
            ba[:2 * c.D].reshape(2 * c.DC, 128).T),
    }

    x = np.asarray(x, f32)
    in_maps = []
    for core in range(c.n_cores):
        b, half = core // 2, core % 2
        own = core_rows(c, half)
        peer = core_rows(c, 1 - half)
        perm = np.concatenate([own, peer])
        m = dict(shared)
        m["x"] = np.ascontiguousarray(x[b][perm])
        m["qidx"] = own.astype(f32).reshape(1, c.T)
        kofs = np.empty((128, c.KC), f32)
        for kc in range(c.KC):
            kofs[:, kc] = perm[kc * 128 + np.arange(128)]
        m["kofs"] = kofs
        in_maps.append(m)
    return in_maps


_NC_CACHE = {}


def get_nc(cfg: Cfg):
    key = (cfg.B, cfg.S, cfg.D, cfg.H, cfg.F, cfg.qkv_fp8, cfg.fc_fp8,
           cfg.mproj_fp8, cfg.av_fp8, cfg.BS)
    if key not in _NC_CACHE:
        _NC_CACHE[key] = build(cfg)
    return _NC_CACHE[key]


def kernel(**inputs) -> np.ndarray:
    from concourse.bass_utils import run_bass_kernel_spmd

    cfg = Cfg()
    nc = get_nc(cfg)
    in_maps = make_core_inputs(cfg, **inputs)
    res = run_bass_kernel_spmd(nc, in_maps, core_ids=list(range(cfg.n_cores)))
    B, S, D = cfg.B, cfg.S, cfg.D
    out = np.empty((B, S, D), np.float32)
    for core in range(cfg.n_cores):
        b, half = core // 2, core % 2
        out[b, core_rows(cfg, half), :] = res.results[core]["y"]
    return out


# revision 30
# speedup vs baseline: 2.0606x; 1.0063x over previous
"""Single transformer block on 8 NeuronCores — collective-free.

Sharding: core c = (batch b=c//2, parity p=c%2). Each core receives the FULL
sequence of its batch, permuted to [own-stripe | peer-stripe] order, and
recomputes K and V for all 2048 tokens locally — cheaper than the pairwise
AllGather it replaces (~55us extra PE vs ~270us of collective time) and it
deletes all DRAM bounce traffic.  Q / attention / c_proj / MLP cover only the
core's 1024 own (striped) tokens.

Tricks:
  - LayerNorm affine (w, b) folds host-side into the following matmul
    weights/bias, so on-chip LN is just (x - mean) * rsqrt(var + eps).
  - The 1/sqrt(hd) query scale folds host-side into W_q / b_q.
  - V is built directly in [128 key, KC, H, 65] layout with a ones column at
    65, so AV yields the softmax denominator for free and per-head V slices
    are zero-copy views.
  - Scores are computed transposed S^T[k, q]; causal mask is a 0/1 multiply
    on P = exp(S) (finite, exact).
  - AV accumulates a whole 512-query group into one [65, 512] PSUM bank:
    chunks common to both 256-slots run 512-wide, diagonal-extra chunks run
    256-wide into the upper half.
  - Optional fp8 (e4m3) QKV projection with DoubleRow matmuls (2x PE rate,
    half the instructions); weights are pre-scaled x64 host-side so 0.02-std
    values stay out of the fp8 subnormal range, undone at eviction.
"""

import math
from contextlib import ExitStack

import numpy as np
import ml_dtypes

import concourse.bacc as bacc
import concourse.bass as bass
import concourse.mybir as mybir
import concourse.tile as tile
from concourse.masks import make_identity

F32 = mybir.dt.float32
F32R = mybir.dt.float32r
BF16 = mybir.dt.bfloat16
F8 = mybir.dt.float8e4
AF = mybir.ActivationFunctionType
ALU = mybir.AluOpType

EPS = 1e-5


class Cfg:
    def __init__(self, B=4, S=2048, D=1024, H=16, F=4096, n_cores=8,
                 qkv_fp8=True, fc_fp8=False, mproj_fp8=True, av_fp8=True,
                 BS=128):
        self.B, self.S, self.D, self.H, self.F = B, S, D, H, F
        self.n_cores = n_cores
        assert n_cores == 2 * B
        self.HD = D // H
        assert self.HD == 64
        self.T = S // 2            # tokens owned per core
        self.KT = S // 128         # token 128-tiles, full sequence
        self.TB = self.T // 128    # token 128-tiles, local
        self.DC = D // 128         # contraction chunks over D
        self.QF = 512              # free-dim tile for projection matmuls
        self.KC = S // 128         # key 128-chunks over full sequence
        self.GB = F // 128         # MLP hidden 128-blocks
        self.HPB = 128 // self.HD  # heads per 128-feature block (=2)
        self.BS = BS               # stripe block (q-slot) size
        self.SLOTS = self.T // self.BS
        self.SPG = 512 // self.BS  # slots per 512-wide attention group
        self.KCH = self.KC // 2    # chunks per parity half
        self.CPB = self.BS // 128  # key chunks per stripe block
        self.qkv_fp8 = qkv_fp8
        self.fc_fp8 = fc_fp8
        self.mproj_fp8 = mproj_fp8
        self.av_fp8 = av_fp8
        self.wscale = 64.0 if qkv_fp8 else 1.0
        self.fscale = 64.0 if fc_fp8 else 1.0
        self.mscale = 64.0 if mproj_fp8 else 1.0


def chunk_absblk(c, kc):
    # conservative absolute stripe-block index covered by key chunk kc
    parity = kc // c.KCH
    loc = kc % c.KCH
    return 2 * (loc // (c.BS // 128)) + parity


def build(cfg: Cfg):
    c = cfg
    QT = F8 if c.qkv_fp8 else BF16
    FT = F8 if c.fc_fp8 else BF16
    MPT = F8 if c.mproj_fp8 else BF16
    VT = F8 if c.av_fp8 else BF16
    PT = VT
    nc = bacc.Bacc(None, target_bir_lowering=False)

    # ---------------- I/O ----------------
    x_in = nc.dram_tensor("x", [c.S, c.D], F32, kind="ExternalInput")
    w_attn = nc.dram_tensor("w_attn", [c.D, 3 * c.D], QT, kind="ExternalInput")
    w_cproj = nc.dram_tensor("w_cproj", [c.D, c.D], BF16, kind="ExternalInput")
    w_fc = nc.dram_tensor("w_fc", [c.D, c.F], FT, kind="ExternalInput")
    w_mproj = nc.dram_tensor("w_mproj", [c.F, c.D], MPT,
                             kind="ExternalInput")
    battn_qk_in = nc.dram_tensor("battn_qk", [128, 2 * c.DC], F32,
                                 kind="ExternalInput")
    bv_in = nc.dram_tensor("bv", [1, c.D], F32, kind="ExternalInput")
    bcp_in = nc.dram_tensor("bcp", [1, c.D], F32, kind="ExternalInput")
    bmp_in = nc.dram_tensor("bmp", [1, c.D], F32, kind="ExternalInput")
    bfc_in = nc.dram_tensor("bfc", [128, c.GB], F32, kind="ExternalInput")
    qidx_in = nc.dram_tensor("qidx", [1, c.T], F32, kind="ExternalInput")
    kofs_in = nc.dram_tensor("kofs", [128, c.KC], F32, kind="ExternalInput")
    y_out = nc.dram_tensor("y", [c.T, c.D], F32, kind="ExternalOutput")

    def bcast(dram, p=128):
        # partition-broadcast DMA source: read row 0 for every partition
        return bass.AP(tensor=dram, offset=0, ap=[[0, p], [1, dram.shape[1]]])

    with tile.TileContext(nc) as tc, ExitStack() as es:
        gconst = es.enter_context(tc.tile_pool(name="gconst", bufs=1))
        ident = gconst.tile([128, 128], F32)
        make_identity(nc, ident[:])
        eps_t = gconst.tile([128, 1], F32)
        nc.vector.memset(eps_t[:], EPS)
        ones64_f = gconst.tile([1, 64], F32)
        nc.vector.memset(ones64_f[:], 1.0)
        ones64 = gconst.tile([1, 64], F32R)
        nc.vector.tensor_copy(ones64[:], ones64_f[:])

        def layernorm_to(get_src, n_tiles, dest, lnp, ps_tr, tag,
                         interleave=None):
            """normalize token tiles and write feature-major into dest
            [128, DC, n_tiles*128].  get_src(tb) -> token-major [128, D] tile.
            interleave(g) is called after every 4th tile to emit consumer
            work early (keeps PE fed in emission order)."""
            for tb in range(n_tiles):
                src = get_src(tb)
                st = lnp.tile([128, 2, 6], F32, tag=f"{tag}st")
                for sg in range(2):
                    nc.vector.bn_stats(
                        out=st[:, sg, :], in_=src[:, sg * 512:(sg + 1) * 512])
                mv = lnp.tile([128, 2], F32, tag=f"{tag}mv")
                nc.vector.bn_aggr(out=mv[:], in_=st[:])
                sd = lnp.tile([128, 1], F32, tag=f"{tag}sd")
                nc.scalar.activation(sd[:], mv[:, 1:2], AF.Sqrt,
                                     bias=eps_t[:, 0:1])
                rs = lnp.tile([128, 1], F32, tag=f"{tag}rs")
                nc.vector.reciprocal(rs[:], sd[:])
                nrm = lnp.tile([128, c.D], F32, tag=f"{tag}n")
                nc.vector.tensor_scalar(
                    out=nrm[:], in0=src[:], scalar1=mv[:, 0:1],
                    scalar2=rs[:, 0:1], op0=ALU.subtract, op1=ALU.mult)
                for i2 in range(c.DC // 4):
                    pt = ps_tr.tile([128, 512], F32, tag=f"{tag}tr")
                    for j in range(4):
                        ch = 4 * i2 + j
                        nc.tensor.matmul(
                            pt[:, j * 128:(j + 1) * 128],
                            nrm[:, ch * 128:(ch + 1) * 128], ident[:],
                            is_transpose=True, start=(j == 0), stop=(j == 3))
                    nc.scalar.activation(
                        dest[:, 4 * i2:4 * i2 + 4, tb * 128:(tb + 1) * 128],
                        pt[:], AF.Identity)
                if interleave is not None and tb % 4 == 3:
                    interleave(tb // 4)

        # ---------------- persistent activations ----------------
        es_per = ExitStack()
        xloc = []
        xlp = es_per.enter_context(tc.tile_pool(name="xloc", bufs=1,
                                                side="left"))
        for tb in range(c.TB):
            t = xlp.tile([128, c.D], F32, tag=f"x{tb}", name=f"x{tb}")
            nc.sync.dma_start(out=t[:], in_=x_in[tb * 128:(tb + 1) * 128, :])
            xloc.append(t)

        es_kvq = ExitStack()
        kvqp = es_kvq.enter_context(tc.tile_pool(name="kvq", bufs=1,
                                                 side="right"))
        ktp = kvqp.tile([128, c.DC, c.S], BF16, name="ktp")
        vtt = kvqp.tile([128, 2, c.KCH, c.H, 65], VT, name="vtt")
        qtp = kvqp.tile([128, c.DC, c.T], BF16, name="qtp")
        nc.vector.memset(vtt[:, :, :, :, 64:65], 1.0)

        # ================= phase A: LN1 + QKV =================
        es_ht = ExitStack()
        htp = es_ht.enter_context(tc.tile_pool(name="htp", bufs=1))
        ht = htp.tile([128, c.DC, c.S], QT, name="ht")

        with (
            tc.tile_pool(name="aconst", bufs=1) as aconst,
            tc.tile_pool(name="xs", bufs=2) as xsp,
            tc.tile_pool(name="lnp", bufs=2) as lnp,
            tc.tile_pool(name="wa", bufs=2) as wap,
            tc.tile_pool(name="ko", bufs=4) as kop,
            tc.tile_pool(name="ps_tr", bufs=2, space="PSUM") as ps_tr,
            tc.tile_pool(name="ps_mm", bufs=2, space="PSUM") as psmm,
        ):
            battn_qk = aconst.tile([128, 2 * c.DC], F32)
            nc.sync.dma_start(out=battn_qk[:], in_=battn_qk_in[:, :])
            bv_b = aconst.tile([128, c.D], F32)
            nc.sync.dma_start(out=bv_b[:], in_=bcast(bv_in))
            bcp_b = aconst.tile([128, c.D], F32)
            nc.sync.dma_start(out=bcp_b[:], in_=bcast(bcp_in))

            wk = wap.tile([128, c.DC, c.D], QT, tag="wa", name="wk")
            nc.scalar.dma_start(
                out=wk[:],
                in_=w_attn[:, c.D:2 * c.D].rearrange("(i p) f -> p i f",
                                                     p=128))
            wv = wap.tile([128, c.DC, c.D], QT, tag="wa", name="wv")
            nc.scalar.dma_start(
                out=wv[:],
                in_=w_attn[:, 2 * c.D:3 * c.D].rearrange("(i p) f -> p i f",
                                                         p=128))

            inv_w = 1.0 / c.wscale
            NI = c.DC // 2 if c.qkv_fp8 else c.DC  # contraction steps

            def wsl(wslab, i, fsl):
                # weight slab contraction-step slice (pair of chunks in fp8)
                if c.qkv_fp8:
                    return wslab[:, 2 * i:2 * i + 2, fsl]
                return wslab[:, i, fsl]

            def hsl(i, tsl):
                if c.qkv_fp8:
                    return ht[:, 2 * i:2 * i + 2, tsl]
                return ht[:, i, tsl]

            PM = mybir.MatmulPerfMode.DoubleRow if c.qkv_fp8 else None

            def v_chunk(g):
                # V for these 4 token tiles; lhsT (ht slice) shared across
                # both feature halves so ldweights is loaded once per step
                for tb in range(4 * g, 4 * g + 4):
                    tbs = slice(tb * 128, (tb + 1) * 128)
                    pss = [psmm.tile([128, 512], F32, tag="ps", name=f"psv{q}")
                           for q in range(2)]
                    for i in range(NI):
                        for vh in range(2):
                            nc.tensor.matmul(
                                pss[vh][:], hsl(i, tbs),
                                wsl(wv, i, slice(vh * 512, (vh + 1) * 512)),
                                start=(i == 0), stop=(i == NI - 1),
                                perf_mode=PM)
                    for vh in range(2):
                        fsl = slice(vh * 512, (vh + 1) * 512)
                        dst = vtt[:, tb // c.KCH, tb % c.KCH,
                                  vh * 8:(vh + 1) * 8, 0:64]
                        if c.qkv_fp8:
                            vo = kop.tile([128, 512], BF16, tag="vo")
                            nc.scalar.activation(vo[:], pss[vh][:],
                                                 AF.Identity, scale=inv_w)
                            nc.vector.tensor_add(dst, vo[:], bv_b[:, fsl])
                        else:
                            nc.vector.tensor_add(dst, pss[vh][:],
                                                 bv_b[:, fsl])

            def get_src(tb):
                if tb < c.TB:
                    return xloc[tb]
                t = xsp.tile([128, c.D], F32, tag="xs")
                nc.sync.dma_start(out=t[:],
                                  in_=x_in[tb * 128:(tb + 1) * 128, :])
                return t

            layernorm_to(get_src, c.KT, ht, lnp, ps_tr, "a",
                         interleave=v_chunk)

            # ---- k^T pass: lhsT (w chunk) shared across 4 token slices ----
            for m in range(c.DC):
                msl = slice(m * 128, (m + 1) * 128)
                pss = [psmm.tile([128, 2, 512], F32, tag="pk",
                                 name=f"psk{q}") for q in range(2)]
                for i in range(NI):
                    for th in range(4):
                        nc.tensor.matmul(
                            pss[th // 2][:, th % 2, :], wsl(wk, i, msl),
                            hsl(i, slice(th * 512, (th + 1) * 512)),
                            start=(i == 0), stop=(i == NI - 1),
                            perf_mode=PM)
                for q in range(2):
                    nc.scalar.activation(
                        ktp[:, m, q * 1024:(q + 1) * 1024], pss[q][:],
                        AF.Identity,
                        bias=battn_qk[:, c.DC + m:c.DC + m + 1], scale=inv_w)

            # ---- q^T pass (local tokens only; scale folded host-side) ----
            wq = wap.tile([128, c.DC, c.D], QT, tag="wa", name="wq")
            nc.scalar.dma_start(
                out=wq[:],
                in_=w_attn[:, 0:c.D].rearrange("(i p) f -> p i f", p=128))
            for m in range(c.DC):
                msl = slice(m * 128, (m + 1) * 128)
                psq = psmm.tile([128, 2, 512], F32, tag="pk", name="psq")
                for i in range(NI):
                    for th in range(2):
                        nc.tensor.matmul(
                            psq[:, th, :], wsl(wq, i, msl),
                            hsl(i, slice(th * 512, (th + 1) * 512)),
                            start=(i == 0), stop=(i == NI - 1),
                            perf_mode=PM)
                nc.scalar.activation(
                    qtp[:, m, :], psq[:],
                    AF.Identity, bias=battn_qk[:, m:m + 1], scale=inv_w)

            # fold the c_proj bias into the residual copy of x, in place
            for tb in range(c.TB):
                nc.vector.tensor_add(xloc[tb][:], xloc[tb][:], bcp_b[:])

        es_ht.close()

        # ================= phase B: attention =================
        # prefetch c_proj weights during attention
        es_wc = ExitStack()
        wcp = es_wc.enter_context(tc.tile_pool(name="wc", bufs=1,
                                               side="left"))
        wc = wcp.tile([128, c.DC, c.D], BF16, name="wc")
        nc.scalar.dma_start(
            out=wc[:], in_=w_cproj[:, :].rearrange("(i p) f -> p i f", p=128))

        es_at = ExitStack()
        atp = es_at.enter_context(tc.tile_pool(name="atp", bufs=1,
                                               side="left"))
        at = atp.tile([128, c.DC, c.T], BF16, name="at")

        with (
            tc.tile_pool(name="bconst", bufs=1) as bconst,
            tc.tile_pool(name="mask", bufs=1) as maskp,
            tc.tile_pool(name="pt", bufs=4) as ptp,
            tc.tile_pool(name="rec", bufs=3) as recp,
            tc.tile_pool(name="ps_qk", bufs=2, space="PSUM") as psqk,
            tc.tile_pool(name="ps_o", bufs=3, space="PSUM") as pso,
            tc.tile_pool(name="ps_bc", bufs=1, space="PSUM") as psbc,
        ):
            qidx = bconst.tile([128, c.T], F32)
            nc.sync.dma_start(out=qidx[:], in_=bcast(qidx_in))
            kofs = bconst.tile([128, c.KC], F32)
            nc.sync.dma_start(out=kofs[:], in_=kofs_in[:, :])

            # groups of SPG query slots, 512 queries wide; key-chunk pairs
            # (own-parity loc, peer-parity loc+KCH) share one QK psum, one
            # exp, and (in fp8) one DoubleRow AV matmul.  A chunk with
            # minimal covered slot s_min = loc//CPB only spans query columns
            # [max(s0,s_min)*BS ...), so late chunks run narrow.
            groups = [list(range(c.SPG * gi, c.SPG * (gi + 1)))
                      for gi in range(c.SLOTS // c.SPG)]

            # A chunk's mask differs from all-ones only in the diagonal
            # slot s_min (queries in later slots are >= every key of the
            # chunk for either parity), so one [128, BS] mask per chunk.
            masks = {}
            for kc in range(c.KC):
                s_min = (kc % c.KCH) // c.CPB
                qsl = slice(s_min * c.BS, (s_min + 1) * c.BS)
                mk = maskp.tile([128, c.BS], BF16, tag=f"mk{kc}",
                                name=f"mk{kc}")
                nc.vector.tensor_scalar(
                    out=mk[:], in0=qidx[:, qsl],
                    scalar1=kofs[:, kc:kc + 1], scalar2=None,
                    op0=ALU.is_ge)
                masks[kc] = mk

            def po_slice(po, ocol):
                return po[:, ocol:512]

            # both heads of a feature block run interleaved: two independent
            # QK->exp->AV chains hide each other's cross-engine latencies
            for jj in range(c.DC):
                for gi, g in enumerate(groups):
                    s0, s3 = g[0], g[-1]
                    n_loc = (s3 + 1) * c.CPB
                    pos = [pso.tile([65, 512], F32, tag="po",
                                    name=f"po{hp}") for hp in range(c.HPB)]
                    for loc in range(n_loc):
                        lo = max(s0, loc // c.CPB)
                        w = (s3 - lo + 1) * c.BS
                        ocol = (lo - s0) * c.BS
                        qsl = slice(lo * c.BS, (s3 + 1) * c.BS)
                        for hp in range(c.HPB):
                            h = c.HPB * jj + hp
                            base = hp * 64
                            ps2 = psqk.tile([128, 2, 512], F32, tag="qk")
                            pt = ptp.tile([128, 2, 512], PT, tag="pt")
                            for ix in range(2):
                                kc = loc + ix * c.KCH
                                nc.tensor.matmul(
                                    ps2[:, ix, 0:w],
                                    ktp[base:base + 64, jj,
                                        kc * 128:(kc + 1) * 128],
                                    qtp[base:base + 64, jj, qsl],
                                    start=True, stop=True)
                            nc.scalar.activation(pt[:, :, 0:w],
                                                 ps2[:, :, 0:w], AF.Exp)
                            if loc // c.CPB >= s0:
                                for ix in range(2):
                                    kc = loc + ix * c.KCH
                                    nc.vector.tensor_mul(
                                        pt[:, ix, 0:c.BS],
                                        pt[:, ix, 0:c.BS], masks[kc][:])
                            if c.av_fp8:
                                nc.tensor.matmul(
                                    po_slice(pos[hp], ocol),
                                    vtt[:, :, loc, h, :], pt[:, :, 0:w],
                                    start=(loc == 0),
                                    stop=(loc == n_loc - 1),
                                    perf_mode=mybir.MatmulPerfMode.DoubleRow)
                            else:
                                for ix in range(2):
                                    nc.tensor.matmul(
                                        po_slice(pos[hp], ocol),
                                        vtt[:, ix, loc, h, :],
                                        pt[:, ix, 0:w],
                                        start=(loc == 0 and ix == 0),
                                        stop=(loc == n_loc - 1 and ix == 1))
                    # normalize by softmax denominator (row 64)
                    gq = slice(s0 * c.BS, s0 * c.BS + 512)
                    for hp in range(c.HPB):
                        base = hp * 64
                        po = pos[hp]
                        rec = recp.tile([1, 512], F32R, tag="rec")
                        with nc.allow_low_precision(
                                reason="softmax denom in f32r"):
                            nc.vector.reciprocal(rec[:], po[64:65, :])
                        bc = psbc.tile([64, 512], F32, tag="bc")
                        nc.tensor.matmul(bc[:], ones64[:], rec[:],
                                         start=True, stop=True)
                        bcs = recp.tile([64, 512], F32, tag="bcs")
                        nc.vector.tensor_copy(bcs[:], bc[:])
                        nc.vector.tensor_mul(
                            at[base:base + 64, jj, gq], po[0:64, :], bcs[:])

        es_kvq.close()

        # ================= phase C: c_proj + residual =================
        es_x2 = ExitStack()
        x2p = es_x2.enter_context(tc.tile_pool(name="x2p", bufs=1,
                                               side="right"))
        x2t = []
        with tc.tile_pool(name="ps_c", bufs=4, space="PSUM") as psc:
            for tb in range(c.TB):
                x2 = x2p.tile([128, c.D], F32, tag=f"x2_{tb}",
                              name=f"x2_{tb}")
                pss = [psc.tile([128, 512], F32, tag="ps", name=f"psc{q}") for q in range(2)]
                for i in range(c.DC):
                    for fh in range(2):
                        nc.tensor.matmul(
                            pss[fh][:], at[:, i, tb * 128:(tb + 1) * 128],
                            wc[:, i, fh * 512:(fh + 1) * 512],
                            start=(i == 0), stop=(i == c.DC - 1))
                for fh in range(2):
                    fsl = slice(fh * 512, (fh + 1) * 512)
                    nc.vector.tensor_add(x2[:, fsl], pss[fh][:],
                                         xloc[tb][:, fsl])
                x2t.append(x2)

        es_at.close()
        es_wc.close()
        es_per.close()

        # ================= phase D: LN2 + MLP =================
        with (
            tc.tile_pool(name="dconst", bufs=1) as dconst,
            tc.tile_pool(name="gt", bufs=1) as gtp,
            tc.tile_pool(name="wm", bufs=1) as wmp,
        ):
            bmp_b = dconst.tile([128, c.D], F32)
            nc.sync.dma_start(out=bmp_b[:], in_=bcast(bmp_in))
            bfc = dconst.tile([128, c.GB], F32)
            nc.sync.dma_start(out=bfc[:], in_=bfc_in[:, :])

            PMf = mybir.MatmulPerfMode.DoubleRow if c.fc_fp8 else None
            PMm = mybir.MatmulPerfMode.DoubleRow if c.mproj_fp8 else None
            NI2 = c.DC // 2 if c.fc_fp8 else c.DC    # fc contraction steps
            NG = c.GB // 2 if c.mproj_fp8 else c.GB  # mproj contraction steps
            inv_f = 1.0 / c.fscale
            inv_m = 1.0 / c.mscale
            gt = gtp.tile([128, c.GB, c.T], MPT, name="gt")
            x2b = [gtp.tile([128, c.D], F32, tag=f"x2b{tb}",
                            name=f"x2b{tb}") for tb in range(c.TB)]
            # mproj weights prefetched during LN2/fc on the sync queue so
            # they don't serialize behind the fc slab loads (scalar queue)
            wm_all = []
            for fh in range(2):
                wm = wmp.tile([128, c.GB, 512], MPT, tag=f"wm{fh}",
                              name=f"wm{fh}")
                nc.sync.dma_start(
                    out=wm[:],
                    in_=w_mproj[:, fh * 512:(fh + 1) * 512].rearrange(
                        "(g p) f -> p g f", p=128))
                wm_all.append(wm)
            with (
                tc.tile_pool(name="mtp", bufs=1) as mtp,
                tc.tile_pool(name="lnp2", bufs=2) as lnp2,
                tc.tile_pool(name="wf", bufs=2) as wfp,
                tc.tile_pool(name="ps_tr2", bufs=3, space="PSUM") as ps_tr2,
                tc.tile_pool(name="ps_g", bufs=2, space="PSUM") as psg,
            ):
                mt = mtp.tile([128, c.DC, c.T], FT, name="mt")
                layernorm_to(lambda tb: x2t[tb], c.TB, mt, lnp2, ps_tr2, "d")
                for tb in range(c.TB):
                    nc.vector.tensor_add(x2b[tb][:], x2t[tb][:], bmp_b[:])

                def msl(i, tsl):
                    if c.fc_fp8:
                        return mt[:, 2 * i:2 * i + 2, tsl]
                    return mt[:, i, tsl]

                # ---------------- fc + gelu ----------------
                wf = None
                for gb in range(c.GB):
                    if gb % 4 == 0:
                        wf = wfp.tile([128, c.DC, 512], FT, tag="wf",
                                      name=f"wf{gb}")
                        j = gb // 4
                        nc.scalar.dma_start(
                            out=wf[:],
                            in_=w_fc[:, j * 512:(j + 1) * 512].rearrange(
                                "(i p) f -> p i f", p=128))
                    gl = (gb % 4) * 128
                    ps = psg.tile([128, 1024], F32, tag="ps")
                    for i in range(NI2):
                        wfs = (wf[:, 2 * i:2 * i + 2, gl:gl + 128]
                               if c.fc_fp8 else wf[:, i, gl:gl + 128])
                        for th in range(2):
                            nc.tensor.matmul(
                                ps[:, th * 512:(th + 1) * 512], wfs,
                                msl(i, slice(th * 512, (th + 1) * 512)),
                                start=(i == 0), stop=(i == NI2 - 1),
                                perf_mode=PMf)
                    nc.scalar.activation(
                        gt[:, gb, :], ps[:], AF.Gelu_apprx_tanh,
                        bias=bfc[:, gb:gb + 1], scale=inv_f)

            # ---------------- mproj + residual ----------------
            with (
                tc.tile_pool(name="yout", bufs=3) as yop,
                tc.tile_pool(name="mo", bufs=3) as mop,
                tc.tile_pool(name="ps_m", bufs=4, space="PSUM") as psm,
            ):
                for tb in range(c.TB):
                    tbs = slice(tb * 128, (tb + 1) * 128)
                    yo = yop.tile([128, c.D], F32, tag="yo")
                    pss = [psm.tile([128, 512], F32, tag="ps", name=f"psm{q}")
                           for q in range(2)]
                    for g in range(NG):
                        gts = (gt[:, 2 * g:2 * g + 2, tbs]
                               if c.mproj_fp8 else gt[:, g, tbs])
                        for fh in range(2):
                            wms = (wm_all[fh][:, 2 * g:2 * g + 2, :]
                                   if c.mproj_fp8 else wm_all[fh][:, g, :])
                            nc.tensor.matmul(
                                pss[fh][:], gts, wms,
                                start=(g == 0), stop=(g == NG - 1),
                                perf_mode=PMm)
                    for fh in range(2):
                        fsl = slice(fh * 512, (fh + 1) * 512)
                        if c.mproj_fp8:
                            mo = mop.tile([128, 512], F32, tag="mo")
                            nc.scalar.activation(mo[:], pss[fh][:],
                                                 AF.Identity, scale=inv_m)
                            nc.vector.tensor_add(yo[:, fsl], mo[:],
                                                 x2b[tb][:, fsl])
                        else:
                            nc.vector.tensor_add(yo[:, fsl], pss[fh][:],
                                                 x2b[tb][:, fsl])
                    nc.sync.dma_start(
                        out=y_out[tb * 128:(tb + 1) * 128, :], in_=yo[:])

        es_x2.close()

    nc.compile()
    return nc


def core_rows(cfg, half):
    """absolute sequence rows owned by a core with parity half"""
    c = cfg
    loc = np.arange(c.T)
    return (2 * (loc // c.BS) + half) * c.BS + loc % c.BS


def make_core_inputs(cfg: Cfg, x, ln1_w, ln1_b, W_attn, b_attn, W_cproj,
                     b_cproj, ln2_w, ln2_b, W_fc, b_fc, W_mproj, b_mproj):
    """Split full inputs into one in_map per core."""
    c = cfg
    f32 = np.float32
    qt = ml_dtypes.float8_e4m3fn if c.qkv_fp8 else ml_dtypes.bfloat16

    # fold LN1 affine + query scale into W_attn / b_attn
    ln1_w = np.asarray(ln1_w, f32)
    ln1_b = np.asarray(ln1_b, f32)
    Wa = np.asarray(W_attn, f32) * ln1_w[:, None]
    ba = np.asarray(b_attn, f32) + ln1_b @ np.asarray(W_attn, f32)
    qs = 1.0 / math.sqrt(c.HD)
    Wa = Wa.copy()
    Wa[:, :c.D] *= qs
    ba = ba.copy()
    ba[:c.D] *= qs
    Wa_dev = (Wa * c.wscale).astype(qt)

    # fold LN2 affine into W_fc / b_fc
    ln2_w = np.asarray(ln2_w, f32)
    ln2_b = np.asarray(ln2_b, f32)
    Wf = np.asarray(W_fc, f32) * ln2_w[:, None]
    bf = np.asarray(b_fc, f32) + ln2_b @ np.asarray(W_fc, f32)

    fc_dt = ml_dtypes.float8_e4m3fn if c.fc_fp8 else ml_dtypes.bfloat16
    mp_dt = ml_dtypes.float8_e4m3fn if c.mproj_fp8 else ml_dtypes.bfloat16
    shared = {
        "w_attn": np.ascontiguousarray(Wa_dev),
        "w_cproj": np.ascontiguousarray(W_cproj).astype(ml_dtypes.bfloat16),
        "w_fc": np.ascontiguousarray(Wf * c.fscale).astype(fc_dt),
        "w_mproj": np.ascontiguousarray(
            np.asarray(W_mproj, f32) * c.mscale).astype(mp_dt),
        "bv": np.ascontiguousarray(ba[2 * c.D:3 * c.D]).reshape(1, c.D),
        "bcp": np.ascontiguousarray(b_cproj, f32).reshape(1, c.D),
        "bmp": np.ascontiguousarray(b_mproj, f32).reshape(1, c.D),
        "bfc": np.ascontiguousarray(bf.reshape(c.GB, 128).T),
        "battn_qk": np.ascontiguousarray(
            ba[:2 * c.D].reshape(2 * c.DC, 128).T),
    }

    x = np.asarray(x, f32)
    in_maps = []
    for core in range(c.n_cores):
        b, half = core // 2, core % 2
        own = core_rows(c, half)
        peer = core_rows(c, 1 - half)
        perm = np.concatenate([own, peer])
        m = dict(shared)
        m["x"] = np.ascontiguousarray(x[b][perm])
        m["qidx"] = own.astype(f32).reshape(1, c.T)
        kofs = np.empty((128, c.KC), f32)
        for kc in range(c.KC):
            kofs[:, kc] = perm[kc * 128 + np.arange(128)]
        m["kofs"] = kofs
        in_maps.append(m)
    return in_maps


_NC_CACHE = {}


def get_nc(cfg: Cfg):
    key = (cfg.B, cfg.S, cfg.D, cfg.H, cfg.F, cfg.qkv_fp8, cfg.fc_fp8,
           cfg.mproj_fp8, cfg.av_fp8, cfg.BS)
    if key not in _NC_CACHE:
        _NC_CACHE[key] = build(cfg)
    return _NC_CACHE[key]


def kernel(**inputs) -> np.ndarray:
    from concourse.bass_utils import run_bass_kernel_spmd

    cfg = Cfg()
    nc = get_nc(cfg)
    in_maps = make_core_inputs(cfg, **inputs)
    res = run_bass_kernel_spmd(nc, in_maps, core_ids=list(range(cfg.n_cores)))
    B, S, D = cfg.B, cfg.S, cfg.D
    out = np.empty((B, S, D), np.float32)
    for core in range(cfg.n_cores):
        b, half = core // 2, core % 2
        out[b, core_rows(cfg, half), :] = res.results[core]["y"]
    return out
